# revision 1
# baseline (speedup 1.0000x reference)
"""DeepWuKong GCN (3-layer GCNConv + max/mean pool + FFN) on 8 TRN2 NeuronCores.

Strategy (graph-level data parallelism, per sharding hint):
  - 128 graphs -> 16 graphs/core; each graph padded to 512 node slots
    (= 4 aligned 128-slot blocks), 8192 node slots/core, 65536 global table
    rows.  Per-layer: each core transforms its own slice (z = h @ Wc[l],
    feature-major on chip), transposes to node-major, AllGathers the full
    z table into shared HBM, then processes the edges whose dst it owns:
    dma_gather (custom SWDGE row gather) pulls z[src] rows for 128-edge
    chunks, a norm-weighted one-hot (built on DVE from iota==dstmod) maps
    each chunk into its dst block via a PE matmul accumulated in PSUM,
    and ScalarE applies bias+ReLU into the next feature-major h tile.
  - Segment pooling is per-core local (graph slots are contiguous), FFN is
    two tiny matmuls; host stitches the 8 [16,2] outputs.

Edge schedules (gather index lists, one-hot dst/norm tables, per-block chunk
counts) are built on host from edge_index/batch; the SPMD program shape is
identical on all cores (per-block chunk counts are maxed over cores).
"""
import sys

sys.path.insert(0, "/opt/trn_rl_repo")

import numpy as np

import concourse.bacc as bacc
import concourse.bass as bass
import concourse.mybir as mybir
import concourse.tile as tile
from concourse.bass_utils import run_bass_kernel_spmd

# ---- problem constants (hardcoded per spec) --------------------------------
N_NODES = 50000
N_EDGES = 600000
N_GRAPHS = 128
D = 128
N_LAYERS = 3
N_CORES = 8
GPC = N_GRAPHS // N_CORES      # 16 graphs per core
GSLOT = 512                    # node slots per graph (4 blocks of 128)
NLOC = GPC * GSLOT             # 8192 node slots per core
NBLK = NLOC // 128             # 64 blocks per core
TOT = N_CORES * NLOC           # 65536 table rows
SPLIT = 32768                  # int16 gather index split
BPG = GSLOT // 128             # blocks per graph

F32 = mybir.dt.float32
I16 = mybir.dt.int16

# SWDGE tuning: a single dma_gather is limited to ~1024 indices (Q7-local
# idx scratch; exceeding it crashes the device). One call per (block,
# bucket) keeps calls at K*128 <= 1024 indices.
DMA_SCRATCH = 16384
BLOCKS_PER_CALL = 1            # gather call covers this many dst blocks
N_QUEUES = 4
MAX_IDX_PER_CALL = 1024


# ===========================================================================
# host-side schedule construction
# ===========================================================================
def _build_schedule(x, edge_index, batch):
    x = np.asarray(x, np.float32)
    ei = np.asarray(edge_index).astype(np.int64)
    batch = np.asarray(batch).astype(np.int64)

    counts = np.bincount(batch, minlength=N_GRAPHS)
    assert counts.max() <= GSLOT, f"graph too big: {counts.max()}"

    deg = np.bincount(ei[1], minlength=N_NODES).astype(np.float64) + 1.0
    dis = 1.0 / np.sqrt(deg)

    graph_start = np.zeros(N_GRAPHS + 1, np.int64)
    np.cumsum(counts, out=graph_start[1:])

    # degree-balanced placement of each graph's nodes into its BPG blocks
    newidx = np.full(N_NODES, -1, np.int64)
    for g in range(N_GRAPHS):
        nodes = np.arange(graph_start[g], graph_start[g + 1])
        if len(nodes) == 0:
            continue
        order = np.argsort(-deg[nodes], kind="stable")
        base = (g // GPC) * NLOC + (g % GPC) * GSLOT
        bin_load = np.zeros(BPG)
        bin_fill = np.zeros(BPG, np.int64)
        for n in nodes[order]:
            cand = np.argsort(bin_load, kind="stable")
            for b in cand:
                if bin_fill[b] < 128:
                    break
            newidx[n] = base + b * 128 + bin_fill[b]
            bin_fill[b] += 1
            bin_load[b] += deg[n]
    assert (newidx[batch >= 0] >= 0).all()

    # edge list with self loops, keyed by owner core of dst
    src = np.concatenate([ei[0], np.arange(N_NODES, dtype=np.int64)])
    dst = np.concatenate([ei[1], np.arange(N_NODES, dtype=np.int64)])
    w = (dis[src] * dis[dst]).astype(np.float32)
    psrc = newidx[src]
    pdst = newidx[dst]
    core = pdst // NLOC
    ldst = pdst % NLOC
    blk = ldst // 128
    hi = (psrc >= SPLIT).astype(np.int64)

    cnt = np.zeros((N_CORES, NBLK, 2), np.int64)
    np.add.at(cnt, (core, blk, hi), 1)
    need = -(-cnt // 128)
    K = need.max(axis=0)                       # [NBLK, 2], same on all cores
    K_lo = K[:, 0].astype(int)
    K_hi = K[:, 1].astype(int)
    assert (K_lo + K_hi > 0).all(), "empty block (tiny graph?)"
    assert K_lo.max() * 128 <= 1024 and K_hi.max() * 128 <= 1024, \
        f"gather call too big: K_lo={K_lo.max()} K_hi={K_hi.max()}"
    NCH = int((K_lo + K_hi).sum())

    lo_off = np.zeros(NBLK + 1, np.int64)
    np.cumsum(K_lo * 128, out=lo_off[1:])
    hi_off = np.zeros(NBLK + 1, np.int64)
    np.cumsum(K_hi * 128, out=hi_off[1:])
    ch_off = np.zeros(NBLK + 1, np.int64)
    np.cumsum(K_lo + K_hi, out=ch_off[1:])
    nlo_slots = int(lo_off[-1])
    nhi_slots = int(hi_off[-1])

    idx_lo = np.zeros((N_CORES, nlo_slots), np.int16)
    idx_hi = np.zeros((N_CORES, nhi_slots), np.int16)
    dstmod = np.full((N_CORES, 128, NCH), -1.0, np.float32)
    normv = np.zeros((N_CORES, 128, NCH), np.float32)

    # vectorized per-(core,blk,bucket) slot assignment
    sort = np.lexsort((hi, blk, core))
    s_core, s_blk, s_hi = core[sort], blk[sort], hi[sort]
    s_ps, s_ld, s_w = psrc[sort], ldst[sort], w[sort]
    gid = (s_core * NBLK + s_blk) * 2 + s_hi
    first = np.ones(len(gid), bool)
    first[1:] = gid[1:] != gid[:-1]
    gstart = np.zeros(len(gid), np.int64)
    idxs_first = np.flatnonzero(first)
    gstart[idxs_first] = idxs_first
    gstart = np.maximum.accumulate(gstart)
    pos = np.arange(len(gid)) - gstart                  # within-group position

    slot = np.where(s_hi == 0, lo_off[s_blk], hi_off[s_blk]) + pos
    chcol = np.where(s_hi == 0, ch_off[s_blk], ch_off[s_blk] + K_lo[s_blk]) \
        + pos // 128
    part = pos % 128
    val = np.where(s_hi == 0, s_ps, s_ps - SPLIT).astype(np.int16)
    lom = s_hi == 0
    idx_lo[s_core[lom], slot[lom]] = val[lom]
    idx_hi[s_core[~lom], slot[~lom]] = val[~lom]
    dstmod[s_core, part, chcol] = (s_ld % 128).astype(np.float32)
    normv[s_core, part, chcol] = s_w

    def wrap_idx(a):                 # [slots] -> [128, slots/16], 8x replicated
        w16 = a.reshape(-1, 16).T
        return np.tile(w16, (8, 1)).copy()

    idx_lo_w = np.stack([wrap_idx(idx_lo[c]) for c in range(N_CORES)])
    idx_hi_w = np.stack([wrap_idx(idx_hi[c]) for c in range(N_CORES)])

    xpad = np.zeros((TOT, D), np.float32)
    xpad[newidx] = x
    x_fm = np.stack([xpad[c * NLOC:(c + 1) * NLOC].T.copy()
                     for c in range(N_CORES)])

    invcnt = (1.0 / np.maximum(counts, 1)).astype(np.float32)
    invcnt_rep = np.stack([
        np.tile(invcnt[c * GPC:(c + 1) * GPC], (128, 1)) for c in range(N_CORES)
    ]).astype(np.float32)

    return dict(
        K_lo=K_lo, K_hi=K_hi, NCH=NCH,
        nlo_slots=nlo_slots, nhi_slots=nhi_slots,
        lo_off=lo_off, hi_off=hi_off, ch_off=ch_off,
        idx_lo=idx_lo_w, idx_hi=idx_hi_w,
        dstmod=dstmod, normv=normv,
        x_fm=x_fm, invcnt_rep=invcnt_rep,
    )


# ===========================================================================
# device kernel
# ===========================================================================
def _build_kernel(sch):
    K_lo, K_hi = sch["K_lo"], sch["K_hi"]
    lo_off, hi_off, ch_off = sch["lo_off"], sch["hi_off"], sch["ch_off"]
    NCH = sch["NCH"]
    NLO16 = sch["nlo_slots"] // 16
    NHI16 = sch["nhi_slots"] // 16

    nc = bacc.Bacc(
        "TRN2",
        target_bir_lowering=False,
        debug=False,
        num_devices=N_CORES,
        num_swdge_queues=N_QUEUES,
        dynamic_dma_scratch_size=DMA_SCRATCH,
    )

    xfm_d = nc.dram_tensor("xfm", [128, NLOC], F32, kind="ExternalInput")
    wc_d = nc.dram_tensor("wc", [N_LAYERS, 128, 128], F32, kind="ExternalInput")
    bct_d = nc.dram_tensor("bct", [128, N_LAYERS], F32, kind="ExternalInput")
    wffn_d = nc.dram_tensor("wffn", [256, 128], F32, kind="ExternalInput")
    bffnt_d = nc.dram_tensor("bffnt", [128, 1], F32, kind="ExternalInput")
    wfin_d = nc.dram_tensor("wfin", [128, 2], F32, kind="ExternalInput")
    bfinr_d = nc.dram_tensor("bfinr", [GPC, 2], F32, kind="ExternalInput")
    idxlo_d = nc.dram_tensor("idxlo", [128, NLO16], I16, kind="ExternalInput")
    idxhi_d = nc.dram_tensor("idxhi", [128, NHI16], I16, kind="ExternalInput")
    dstmod_d = nc.dram_tensor("dstmod", [128, NCH], F32, kind="ExternalInput")
    normv_d = nc.dram_tensor("normv", [128, NCH], F32, kind="ExternalInput")
    invc_d = nc.dram_tensor("invc", [128, GPC], F32, kind="ExternalInput")
    iota_d = nc.dram_tensor("iota", [128, 128], F32, kind="ExternalInput")
    ident_d = nc.dram_tensor("ident", [128, 128], F32, kind="ExternalInput")
    out_d = nc.dram_tensor("out", [GPC, 2], F32, kind="ExternalOutput")

    RG = [list(range(N_CORES))]

    with tile.TileContext(nc) as tc:
        with (
            tc.tile_pool(name="consts", bufs=1) as consts,
            tc.tile_pool(name="hpool", bufs=2) as hpool,
            tc.tile_pool(name="zpool", bufs=2) as zpool,
            tc.tile_pool(name="gpool", bufs=2) as gpool,
            tc.tile_pool(name="ohpool", bufs=4) as ohpool,
            tc.tile_pool(name="spool", bufs=1) as spool,
            tc.tile_pool(name="ps512", bufs=2, space="PSUM") as ps512,
            tc.tile_pool(name="ps128", bufs=2, space="PSUM") as ps128,
            tc.tile_pool(name="psagg", bufs=2, space="PSUM") as psagg,
            tc.tile_pool(name="psfin", bufs=1, space="PSUM") as psfin,
            tc.tile_pool(name="dram", bufs=1, space="DRAM") as dram,
        ):
            # ---- load constants -------------------------------------------
            wc_sb = consts.tile([128, N_LAYERS, 128], F32)
            nc.sync.dma_start(wc_sb[:], wc_d[:].rearrange("l p f -> p l f"))
            bct_sb = consts.tile([128, N_LAYERS], F32)
            nc.sync.dma_start(bct_sb[:], bct_d[:])
            wffn_sb = consts.tile([128, 2, 128], F32)
            nc.sync.dma_start(
                wffn_sb[:], wffn_d[:].rearrange("(h p) f -> p h f", p=128))
            bffnt_sb = consts.tile([128, 1], F32)
            nc.sync.dma_start(bffnt_sb[:], bffnt_d[:])
            wfin_sb = consts.tile([128, 2], F32)
            nc.sync.dma_start(wfin_sb[:], wfin_d[:])
            bfinr_sb = consts.tile([GPC, 2], F32)
            nc.sync.dma_start(bfinr_sb[:], bfinr_d[:])
            idxlo_sb = consts.tile([128, NLO16], I16)
            nc.sync.dma_start(idxlo_sb[:], idxlo_d[:])
            idxhi_sb = consts.tile([128, NHI16], I16)
            nc.sync.dma_start(idxhi_sb[:], idxhi_d[:])
            dstmod_sb = consts.tile([128, NCH], F32)
            nc.sync.dma_start(dstmod_sb[:], dstmod_d[:])
            normv_sb = consts.tile([128, NCH], F32)
            nc.sync.dma_start(normv_sb[:], normv_d[:])
            invc_sb = consts.tile([128, GPC], F32)
            nc.sync.dma_start(invc_sb[:], invc_d[:])
            iota_sb = consts.tile([128, 128], F32)
            nc.sync.dma_start(iota_sb[:], iota_d[:])
            ident_sb = consts.tile([128, 128], F32)
            nc.sync.dma_start(ident_sb[:], ident_d[:])

            h_cur = hpool.tile([128, NLOC], F32, tag="h", name="h_init")
            nc.sync.dma_start(h_cur[:], xfm_d[:])

            for l in range(N_LAYERS):
                # ---- transform own slice: z[fo, n] = sum_fi Wc[fi,fo] h[fi,n]
                z_own = dram.tile([NLOC, 128], F32, tag="zown", bufs=2,
                                  name=f"zown{l}")
                for g in range(GPC):
                    zps = ps512.tile([128, 512], F32, tag="zps",
                                     name=f"zps{l}_{g}")
                    nc.tensor.matmul(
                        zps[:], wc_sb[:, l, :],
                        h_cur[:, g * 512:(g + 1) * 512],
                        start=True, stop=True)
                    zsb = zpool.tile([128, 512], F32, tag="zsb",
                                     name=f"zsb{l}_{g}")
                    nc.vector.tensor_copy(zsb[:], zps[:])
                    zst = zpool.tile([128, 4, 128], F32, tag="zst",
                                     name=f"zst{l}_{g}")
                    for b in range(4):
                        tps = ps128.tile([128, 128], F32, tag="tps",
                                         name=f"tps{l}_{g}_{b}")
                        nc.tensor.transpose(
                            tps[:], zsb[:, b * 128:(b + 1) * 128], ident_sb[:])
                        nc.vector.tensor_copy(zst[:, b, :], tps[:])
                    nc.sync.dma_start(
                        z_own[g * 512:(g + 1) * 512, :].rearrange(
                            "(b p) f -> p b f", p=128),
                        zst[:])

                # ---- AllGather the z table --------------------------------
                z_full = dram.tile([TOT, 128], F32, tag="zfull", bufs=2,
                                   addr_space="Shared", name=f"zfull{l}")
                nc.gpsimd.collective_compute(
                    "AllGather", mybir.AluOpType.bypass,
                    replica_groups=RG,
                    ins=[z_own[:].opt()],
                    outs=[z_full[:].opt()],
                )

                # ---- gather + aggregate -----------------------------------
                h_nxt = hpool.tile([128, NLOC], F32, tag="h", name=f"h{l + 1}")
                for grp in range(NBLK // BLOCKS_PER_CALL):
                    b0 = grp * BLOCKS_PER_CALL
                    clo = int(sum(K_lo[b0:b0 + BLOCKS_PER_CALL]))
                    chi = int(sum(K_hi[b0:b0 + BLOCKS_PER_CALL]))
                    glo = gpool.tile([128, clo, 128], F32, tag="glo",
                                     name=f"glo{l}_{grp}")
                    c0 = int(lo_off[b0]) // 16
                    nc.gpsimd.dma_gather(
                        glo[:], z_full[0:SPLIT, :],
                        idxlo_sb[:, c0:c0 + clo * 8],
                        num_idxs=clo * 128, num_idxs_reg=clo * 128,
                        elem_size=128, queue_num=(2 * grp) % N_QUEUES,
                    )
                    ghi = gpool.tile([128, chi, 128], F32, tag="ghi",
                                     name=f"ghi{l}_{grp}")
                    c0 = int(hi_off[b0]) // 16
                    nc.gpsimd.dma_gather(
                        ghi[:], z_full[SPLIT:TOT, :],
                        idxhi_sb[:, c0:c0 + chi * 8],
                        num_idxs=chi * 128, num_idxs_reg=chi * 128,
                        elem_size=128, queue_num=(2 * grp + 1) % N_QUEUES,
                    )
                    lbase = 0
                    hbase = 0
                    for b in range(b0, b0 + BLOCKS_PER_CALL):
                        ktot = int(K_lo[b] + K_hi[b])
                        ps = psagg.tile([128, 128], F32, tag="aggps",
                                        name=f"agg{l}_{b}")
                        for j in range(ktot):
                            ch = int(ch_off[b]) + j
                            if j < K_lo[b]:
                                msg = glo[:, lbase + j, :]
                            else:
                                msg = ghi[:, hbase + (j - K_lo[b]), :]
                            oh = ohpool.tile([128, 128], F32, tag="oh",
                                             name=f"oh{l}_{b}_{j}")
                            nc.vector.tensor_scalar(
                                oh[:], iota_sb[:],
                                dstmod_sb[:, ch:ch + 1],
                                normv_sb[:, ch:ch + 1],
                                mybir.AluOpType.is_equal,
                                mybir.AluOpType.mult,
                            )
                            nc.tensor.matmul(
                                ps[:], msg, oh[:],
                                start=(j == 0), stop=(j == ktot - 1))
                        lbase += int(K_lo[b])
                        hbase += int(K_hi[b])
                        nc.scalar.activation(
                            h_nxt[:, b * 128:(b + 1) * 128], ps[:],
                            mybir.ActivationFunctionType.Relu,
                            bias=bct_sb[:, l:l + 1])
                h_cur = h_nxt

            # ---- pooling + FFN --------------------------------------------
            mx = spool.tile([128, GPC], F32)
            sm = spool.tile([128, GPC], F32)
            for g in range(GPC):
                nc.vector.tensor_reduce(
                    mx[:, g:g + 1], h_cur[:, g * GSLOT:(g + 1) * GSLOT],
                    mybir.AxisListType.X, mybir.AluOpType.max)
                nc.vector.tensor_reduce(
                    sm[:, g:g + 1], h_cur[:, g * GSLOT:(g + 1) * GSLOT],
                    mybir.AxisListType.X, mybir.AluOpType.add)
            mean = spool.tile([128, GPC], F32)
            nc.vector.tensor_tensor(
                mean[:], sm[:], invc_sb[:], mybir.AluOpType.mult)

            p1 = psfin.tile([128, GPC], F32, tag="p1")
            nc.tensor.matmul(p1[:], wffn_sb[:, 0, :], mx[:],
                             start=True, stop=False)
            nc.tensor.matmul(p1[:], wffn_sb[:, 1, :], mean[:],
                             start=False, stop=True)
            o1 = spool.tile([128, GPC], F32)
            nc.scalar.activation(
                o1[:], p1[:], mybir.ActivationFunctionType.Relu,
                bias=bffnt_sb[:, 0:1])

            p2 = psfin.tile([GPC, 2], F32, tag="p2")
            nc.tensor.matmul(p2[:], o1[:], wfin_sb[:], start=True, stop=True)
            osb = spool.tile([GPC, 2], F32)
            nc.vector.tensor_tensor(
                osb[:], p2[:], bfinr_sb[:], mybir.AluOpType.add)
            nc.sync.dma_start(out_d[:], osb[:])

    nc.compile()
    return nc


# ===========================================================================
# entry point
# ===========================================================================
_CACHE = {}


def kernel(x, Wc, bc, W_ffn, b_ffn, W_fin, b_fin, edge_index, batch):
    x = np.ascontiguousarray(np.asarray(x, np.float32))
    Wc = np.ascontiguousarray(np.asarray(Wc, np.float32))
    bc = np.ascontiguousarray(np.asarray(bc, np.float32))
    W_ffn = np.ascontiguousarray(np.asarray(W_ffn, np.float32))
    b_ffn = np.ascontiguousarray(np.asarray(b_ffn, np.float32))
    W_fin = np.ascontiguousarray(np.asarray(W_fin, np.float32))
    b_fin = np.ascontiguousarray(np.asarray(b_fin, np.float32))

    sch = _build_schedule(x, edge_index, batch)

    key = (sch["NCH"], sch["nlo_slots"], sch["nhi_slots"],
           tuple(sch["K_lo"]), tuple(sch["K_hi"]))
    if key not in _CACHE:
        _CACHE.clear()
        _CACHE[key] = _build_kernel(sch)
    nc = _CACHE[key]

    iota = np.tile(np.arange(128, dtype=np.float32)[None, :], (128, 1))
    ident = np.eye(128, dtype=np.float32)
    bct = bc.T.copy()                       # [128, 3]
    bffnt = b_ffn[:, None].copy()           # [128, 1]
    bfinr = np.tile(b_fin[None, :], (GPC, 1)).astype(np.float32)

    in_maps = []
    for c in range(N_CORES):
        in_maps.append({
            "xfm": sch["x_fm"][c],
            "wc": Wc, "bct": bct, "wffn": W_ffn, "bffnt": bffnt,
            "wfin": W_fin, "bfinr": bfinr,
            "idxlo": sch["idx_lo"][c], "idxhi": sch["idx_hi"][c],
            "dstmod": sch["dstmod"][c], "normv": sch["normv"][c],
            "invc": sch["invcnt_rep"][c],
            "iota": iota, "ident": ident,
        })

    _CACHE["in_maps"] = in_maps
    res = run_bass_kernel_spmd(nc, in_maps, core_ids=list(range(N_CORES)))
    out = np.concatenate([res.results[c]["out"] for c in range(N_CORES)], 0)
    return out.astype(np.float32)


def timed_run(inputs=None):
    """Re-run the cached compiled kernel with profiling; returns exec ns."""
    import time
    nc = next(v for k, v in _CACHE.items() if k != "in_maps")
    in_maps = _CACHE["in_maps"]
    # warm re-runs for a wall-clock floor estimate
    walls = []
    for _ in range(3):
        t0 = time.time()
        run_bass_kernel_spmd(nc, in_maps, core_ids=list(range(N_CORES)))
        walls.append(time.time() - t0)
    print(f"warm re-run walls: {[f'{w*1e3:.1f}ms' for w in walls]}")
    try:
        res = run_bass_kernel_spmd(
            nc, in_maps, core_ids=list(range(N_CORES)), trace=True)
        if res.exec_time_ns is not None:
            return res.exec_time_ns
    except Exception as e:
        print(f"(ntff profiling unavailable: {type(e).__name__}; "
              f"reporting warm wall-clock upper bound)")
    return int(min(walls) * 1e9)


if __name__ == "__main__":
    rng = np.random.default_rng(0)
    x = rng.standard_normal((N_NODES, D), dtype=np.float32)
    ei = rng.integers(0, N_NODES, (2, N_EDGES)).astype(np.int64)
    batch = np.sort(rng.integers(0, N_GRAPHS, N_NODES)).astype(np.int64)
    Wc = rng.standard_normal((3, D, D), dtype=np.float32) * 0.05
    out = kernel(x, Wc, np.zeros((3, D), np.float32),
                 rng.standard_normal((2 * D, D), dtype=np.float32) * 0.05,
                 np.zeros((D,), np.float32),
                 rng.standard_normal((D, 2), dtype=np.float32) * 0.05,
                 np.zeros((2,), np.float32), ei, batch)
    print(out.shape, out[:4])



# revision 4
# speedup vs baseline: 1.7360x; 1.7360x over previous
"""DeepWuKong GCN (3-layer GCNConv + max/mean pool + FFN) on 8 TRN2 NeuronCores.

Strategy (graph-level data parallelism, per sharding hint):
  - 128 graphs -> 16 graphs/core; each graph padded to 512 node slots
    (= 4 aligned 128-slot blocks), 8192 node slots/core, 65536 global table
    rows.  Per-layer: each core transforms its own slice (z = h @ Wc[l],
    feature-major on chip, bf16), transposes to node-major, AllGathers the
    full bf16 z table into shared HBM, then processes the edges whose dst it
    owns: dma_gather (SWDGE row gather, 256B bf16 rows) pulls z[src] rows in
    uniform 1024-index calls, a norm-weighted one-hot (built on DVE from
    iota==dstmod, bf16) maps each 128-edge chunk into its dst block via a PE
    matmul accumulated in fp32 PSUM, and ScalarE applies bias+ReLU into the
    next bf16 feature-major h tile.
  - Self-loops (the PyG-added one plus any data self-edges) never touch the
    gather path: they are one local matmul per dst block against a constant
    diagonal weight matrix read from the node-major z kept in SBUF.
  - Segment pooling is per-core local (graph slots are contiguous), FFN is
    two tiny matmuls; host stitches the 8 [16,2] outputs.

Everything on the message path is bf16 (z table, gathers, one-hots,
matmul operands); accumulation stays fp32 in PSUM.  Gather calls are
decoupled from dst-block boundaries: each SWDGE call covers 8 chunks
(1024 indices, the per-call limit) to amortize the ~1us fixed Q7 cost.
"""
import sys

sys.path.insert(0, "/opt/trn_rl_repo")

import numpy as np
import ml_dtypes

import concourse.bacc as bacc
import concourse.bass as bass
import concourse.mybir as mybir
import concourse.tile as tile
from concourse.bass_utils import run_bass_kernel_spmd

BF16NP = np.dtype(ml_dtypes.bfloat16)

# ---- problem constants (hardcoded per spec) --------------------------------
N_NODES = 50000
N_EDGES = 600000
N_GRAPHS = 128
D = 128
N_LAYERS = 3
N_CORES = 8
GPC = N_GRAPHS // N_CORES      # 16 graphs per core
GSLOT = 512                    # node slots per graph (4 blocks of 128)
NLOC = GPC * GSLOT             # 8192 node slots per core
NBLK = NLOC // 128             # 64 blocks per core
TOT = N_CORES * NLOC           # 65536 table rows
SPLIT = 32768                  # int16 gather index split
BPG = GSLOT // 128             # blocks per graph

F32 = mybir.dt.float32
BF16 = mybir.dt.bfloat16
I16 = mybir.dt.int16

# SWDGE tuning: a single dma_gather is limited to ~1024 indices (Q7-local
# idx scratch; exceeding it crashes the device).
DMA_SCRATCH = 16384
CHUNKS_PER_CALL = 8            # 8 chunks x 128 idx = 1024 idx per call
N_QUEUES = 4


# ===========================================================================
# host-side schedule construction
# ===========================================================================
def _build_schedule(x, edge_index, batch):
    x = np.asarray(x, np.float32)
    ei = np.asarray(edge_index).astype(np.int64)
    batch = np.asarray(batch).astype(np.int64)

    counts = np.bincount(batch, minlength=N_GRAPHS)
    assert counts.max() <= GSLOT, f"graph too big: {counts.max()}"

    deg = np.bincount(ei[1], minlength=N_NODES).astype(np.float64) + 1.0
    dis = 1.0 / np.sqrt(deg)

    graph_start = np.zeros(N_GRAPHS + 1, np.int64)
    np.cumsum(counts, out=graph_start[1:])

    # degree-balanced placement of each graph's nodes into its BPG blocks
    newidx = np.full(N_NODES, -1, np.int64)
    for g in range(N_GRAPHS):
        nodes = np.arange(graph_start[g], graph_start[g + 1])
        if len(nodes) == 0:
            continue
        order = np.argsort(-deg[nodes], kind="stable")
        base = (g // GPC) * NLOC + (g % GPC) * GSLOT
        bin_load = np.zeros(BPG)
        bin_fill = np.zeros(BPG, np.int64)
        for n in nodes[order]:
            cand = np.argsort(bin_load, kind="stable")
            for b in cand:
                if bin_fill[b] < 128:
                    break
            newidx[n] = base + b * 128 + bin_fill[b]
            bin_fill[b] += 1
            bin_load[b] += deg[n]
    assert (newidx[batch >= 0] >= 0).all()

    # self weights: the PyG-added loop plus any data self-edges, all with
    # weight dis[n]^2; these go through the local diag matmul, not gathers
    selfmask = ei[0] == ei[1]
    nself = np.bincount(ei[1][selfmask], minlength=N_NODES)
    wself = (1.0 + nself) * dis * dis

    diag = np.zeros((N_CORES, 128, NBLK, 128), np.float32)
    pall = newidx
    cc = pall // NLOC
    bb = (pall % NLOC) // 128
    ss = pall % 128
    diag[cc, ss, bb, ss] = wself.astype(np.float32)

    # non-self edge list, keyed by owner core of dst
    src = ei[0][~selfmask]
    dst = ei[1][~selfmask]
    w = (dis[src] * dis[dst]).astype(np.float32)
    psrc = newidx[src]
    pdst = newidx[dst]
    core = pdst // NLOC
    ldst = pdst % NLOC
    blk = ldst // 128
    hi = (psrc >= SPLIT).astype(np.int64)

    cnt = np.zeros((N_CORES, NBLK, 2), np.int64)
    np.add.at(cnt, (core, blk, hi), 1)
    need = -(-cnt // 128)
    K = need.max(axis=0)                       # [NBLK, 2], same on all cores
    K_lo = K[:, 0].astype(int)
    K_hi = K[:, 1].astype(int)
    NCH_lo = int(K_lo.sum())
    NCH_hi = int(K_hi.sum())
    # pad chunk counts to a multiple of CHUNKS_PER_CALL for uniform calls
    PCH_lo = -(-NCH_lo // CHUNKS_PER_CALL) * CHUNKS_PER_CALL
    PCH_hi = -(-NCH_hi // CHUNKS_PER_CALL) * CHUNKS_PER_CALL

    lo_ch_off = np.zeros(NBLK + 1, np.int64)
    np.cumsum(K_lo, out=lo_ch_off[1:])
    hi_ch_off = np.zeros(NBLK + 1, np.int64)
    np.cumsum(K_hi, out=hi_ch_off[1:])

    idx_lo = np.zeros((N_CORES, PCH_lo * 128), np.int16)
    idx_hi = np.zeros((N_CORES, PCH_hi * 128), np.int16)
    dml = np.full((N_CORES, 128, PCH_lo), -1.0, np.float32)
    nvl = np.zeros((N_CORES, 128, PCH_lo), np.float32)
    dmh = np.full((N_CORES, 128, PCH_hi), -1.0, np.float32)
    nvh = np.zeros((N_CORES, 128, PCH_hi), np.float32)

    # vectorized per-(core,blk,bucket) slot assignment
    sort = np.lexsort((hi, blk, core))
    s_core, s_blk, s_hi = core[sort], blk[sort], hi[sort]
    s_ps, s_ld, s_w = psrc[sort], ldst[sort], w[sort]
    gid = (s_core * NBLK + s_blk) * 2 + s_hi
    first = np.ones(len(gid), bool)
    first[1:] = gid[1:] != gid[:-1]
    gstart = np.zeros(len(gid), np.int64)
    idxs_first = np.flatnonzero(first)
    gstart[idxs_first] = idxs_first
    gstart = np.maximum.accumulate(gstart)
    pos = np.arange(len(gid)) - gstart                  # within-group position

    chcol = np.where(s_hi == 0, lo_ch_off[s_blk], hi_ch_off[s_blk]) \
        + pos // 128
    slot = chcol * 128 + pos % 128
    part = pos % 128
    val = np.where(s_hi == 0, s_ps, s_ps - SPLIT).astype(np.int16)
    lom = s_hi == 0
    idx_lo[s_core[lom], slot[lom]] = val[lom]
    idx_hi[s_core[~lom], slot[~lom]] = val[~lom]
    dml[s_core[lom], part[lom], chcol[lom]] = (s_ld[lom] % 128).astype(np.float32)
    nvl[s_core[lom], part[lom], chcol[lom]] = s_w[lom]
    dmh[s_core[~lom], part[~lom], chcol[~lom]] = (s_ld[~lom] % 128).astype(np.float32)
    nvh[s_core[~lom], part[~lom], chcol[~lom]] = s_w[~lom]

    def wrap_idx(a):                 # [slots] -> [128, slots/16], 8x replicated
        w16 = a.reshape(-1, 16).T
        return np.tile(w16, (8, 1)).copy()

    idx_lo_w = np.stack([wrap_idx(idx_lo[c]) for c in range(N_CORES)])
    idx_hi_w = np.stack([wrap_idx(idx_hi[c]) for c in range(N_CORES)])

    xpad = np.zeros((TOT, D), np.float32)
    xpad[newidx] = x
    x_fm = np.stack([xpad[c * NLOC:(c + 1) * NLOC].T.copy()
                     for c in range(N_CORES)])

    invcnt = (1.0 / np.maximum(counts, 1)).astype(np.float32)
    invcnt_rep = np.stack([
        np.tile(invcnt[c * GPC:(c + 1) * GPC], (128, 1)) for c in range(N_CORES)
    ]).astype(np.float32)

    return dict(
        K_lo=K_lo, K_hi=K_hi,
        NCH_lo=NCH_lo, NCH_hi=NCH_hi, PCH_lo=PCH_lo, PCH_hi=PCH_hi,
        lo_ch_off=lo_ch_off, hi_ch_off=hi_ch_off,
        idx_lo=idx_lo_w, idx_hi=idx_hi_w,
        dml=dml, nvl=nvl, dmh=dmh, nvh=nvh, diag=diag,
        x_fm=x_fm, invcnt_rep=invcnt_rep,
    )


# ===========================================================================
# device kernel
# ===========================================================================
def _build_kernel(sch):
    K_lo, K_hi = sch["K_lo"], sch["K_hi"]
    lo_ch_off, hi_ch_off = sch["lo_ch_off"], sch["hi_ch_off"]
    PCH_lo, PCH_hi = sch["PCH_lo"], sch["PCH_hi"]
    NCALL_lo = PCH_lo // CHUNKS_PER_CALL
    NCALL_hi = PCH_hi // CHUNKS_PER_CALL

    nc = bacc.Bacc(
        "TRN2",
        target_bir_lowering=False,
        debug=False,
        num_devices=N_CORES,
        num_swdge_queues=N_QUEUES,
        dynamic_dma_scratch_size=DMA_SCRATCH,
    )

    xfm_d = nc.dram_tensor("xfm", [128, NLOC], BF16, kind="ExternalInput")
    wc_d = nc.dram_tensor("wc", [N_LAYERS, 128, 128], BF16, kind="ExternalInput")
    bct_d = nc.dram_tensor("bct", [128, N_LAYERS], F32, kind="ExternalInput")
    wffn_d = nc.dram_tensor("wffn", [256, 128], BF16, kind="ExternalInput")
    bffnt_d = nc.dram_tensor("bffnt", [128, 1], F32, kind="ExternalInput")
    wfin_d = nc.dram_tensor("wfin", [128, 2], BF16, kind="ExternalInput")
    bfinr_d = nc.dram_tensor("bfinr", [GPC, 2], F32, kind="ExternalInput")
    idxlo_d = nc.dram_tensor("idxlo", [128, PCH_lo * 8], I16, kind="ExternalInput")
    idxhi_d = nc.dram_tensor("idxhi", [128, PCH_hi * 8], I16, kind="ExternalInput")
    dml_d = nc.dram_tensor("dml", [128, PCH_lo], F32, kind="ExternalInput")
    nvl_d = nc.dram_tensor("nvl", [128, PCH_lo], F32, kind="ExternalInput")
    dmh_d = nc.dram_tensor("dmh", [128, PCH_hi], F32, kind="ExternalInput")
    nvh_d = nc.dram_tensor("nvh", [128, PCH_hi], F32, kind="ExternalInput")
    diag_d = nc.dram_tensor("diag", [128, NBLK, 128], BF16, kind="ExternalInput")
    invc_d = nc.dram_tensor("invc", [128, GPC], F32, kind="ExternalInput")
    iota_d = nc.dram_tensor("iota", [128, 128], BF16, kind="ExternalInput")
    ident_d = nc.dram_tensor("ident", [128, 128], BF16, kind="ExternalInput")
    out_d = nc.dram_tensor("out", [GPC, 2], F32, kind="ExternalOutput")

    RG = [list(range(N_CORES))]

    with tile.TileContext(nc) as tc:
        with (
            tc.tile_pool(name="consts", bufs=1) as consts,
            tc.tile_pool(name="hpool", bufs=2) as hpool,
            tc.tile_pool(name="zpool", bufs=2) as zpool,
            tc.tile_pool(name="zstpool", bufs=2) as zstpool,
            tc.tile_pool(name="glopool", bufs=3) as glopool,
            tc.tile_pool(name="ghipool", bufs=3) as ghipool,
            tc.tile_pool(name="ohpool", bufs=6) as ohpool,
            tc.tile_pool(name="spool", bufs=1) as spool,
            tc.tile_pool(name="ps512", bufs=2, space="PSUM") as ps512,
            tc.tile_pool(name="ps128", bufs=2, space="PSUM") as ps128,
            tc.tile_pool(name="psagg", bufs=2, space="PSUM") as psagg,
            tc.tile_pool(name="psfin", bufs=1, space="PSUM") as psfin,
            tc.tile_pool(name="dram", bufs=1, space="DRAM") as dram,
        ):
            # ---- load constants -------------------------------------------
            wc_sb = consts.tile([128, N_LAYERS, 128], BF16)
            nc.sync.dma_start(wc_sb[:], wc_d[:].rearrange("l p f -> p l f"))
            bct_sb = consts.tile([128, N_LAYERS], F32)
            nc.sync.dma_start(bct_sb[:], bct_d[:])
            wffn_sb = consts.tile([128, 2, 128], BF16)
            nc.sync.dma_start(
                wffn_sb[:], wffn_d[:].rearrange("(h p) f -> p h f", p=128))
            bffnt_sb = consts.tile([128, 1], F32)
            nc.sync.dma_start(bffnt_sb[:], bffnt_d[:])
            wfin_sb = consts.tile([128, 2], BF16)
            nc.sync.dma_start(wfin_sb[:], wfin_d[:])
            bfinr_sb = consts.tile([GPC, 2], F32)
            nc.sync.dma_start(bfinr_sb[:], bfinr_d[:])
            idxlo_sb = consts.tile([128, PCH_lo * 8], I16)
            nc.sync.dma_start(idxlo_sb[:], idxlo_d[:])
            idxhi_sb = consts.tile([128, PCH_hi * 8], I16)
            nc.sync.dma_start(idxhi_sb[:], idxhi_d[:])
            dml_sb = consts.tile([128, PCH_lo], F32)
            nc.sync.dma_start(dml_sb[:], dml_d[:])
            nvl_sb = consts.tile([128, PCH_lo], F32)
            nc.sync.dma_start(nvl_sb[:], nvl_d[:])
            dmh_sb = consts.tile([128, PCH_hi], F32)
            nc.sync.dma_start(dmh_sb[:], dmh_d[:])
            nvh_sb = consts.tile([128, PCH_hi], F32)
            nc.sync.dma_start(nvh_sb[:], nvh_d[:])
            diag_sb = consts.tile([128, NBLK, 128], BF16)
            nc.sync.dma_start(diag_sb[:], diag_d[:])
            invc_sb = consts.tile([128, GPC], F32)
            nc.sync.dma_start(invc_sb[:], invc_d[:])
            iota_sb = consts.tile([128, 128], BF16)
            nc.sync.dma_start(iota_sb[:], iota_d[:])
            ident_sb = consts.tile([128, 128], BF16)
            nc.sync.dma_start(ident_sb[:], ident_d[:])

            h_cur = hpool.tile([128, NLOC], BF16, tag="h", name="h_init")
            nc.sync.dma_start(h_cur[:], xfm_d[:])

            qctr = [0]

            def next_q():
                q = qctr[0] % N_QUEUES
                qctr[0] += 1
                return q

            for l in range(N_LAYERS):
                # ---- transform own slice: z[fo, n] = sum_fi Wc[fi,fo] h[fi,n]
                zst = zstpool.tile([128, NBLK, 128], BF16, tag="zst",
                                   name=f"zst{l}")
                for g in range(GPC):
                    zps = ps512.tile([128, 512], F32, tag="zps",
                                     name=f"zps{l}_{g}")
                    nc.tensor.matmul(
                        zps[:], wc_sb[:, l, :],
                        h_cur[:, g * 512:(g + 1) * 512],
                        start=True, stop=True)
                    zsb = zpool.tile([128, 512], BF16, tag="zsb",
                                     name=f"zsb{l}_{g}")
                    nc.vector.tensor_copy(zsb[:], zps[:])
                    for t in range(4):
                        tps = ps128.tile([128, 128], BF16, tag="tps",
                                         name=f"tps{l}_{g}_{t}")
                        nc.tensor.transpose(
                            tps[:], zsb[:, t * 128:(t + 1) * 128], ident_sb[:])
                        nc.scalar.activation(
                            zst[:, g * 4 + t, :], tps[:],
                            mybir.ActivationFunctionType.Copy)
                z_own = dram.tile([NLOC, 128], BF16, tag="zown", bufs=2,
                                  name=f"zown{l}")
                nc.sync.dma_start(
                    z_own[:].rearrange("(b p) f -> p b f", p=128), zst[:])

                # ---- AllGather the z table --------------------------------
                z_full = dram.tile([TOT, 128], BF16, tag="zfull", bufs=2,
                                   addr_space="Shared", name=f"zfull{l}")
                nc.gpsimd.collective_compute(
                    "AllGather", mybir.AluOpType.bypass,
                    replica_groups=RG,
                    ins=[z_own[:].opt()],
                    outs=[z_full[:].opt()],
                )

                # ---- gather + aggregate -----------------------------------
                h_nxt = hpool.tile([128, NLOC], BF16, tag="h", name=f"h{l + 1}")
                lo_tiles = {}
                hi_tiles = {}
                lo_next = [0]
                hi_next = [0]

                def issue_lo(upto_chunk, l=l, lo_tiles=lo_tiles, lo_next=lo_next,
                             z_full=z_full):
                    while lo_next[0] * CHUNKS_PER_CALL < upto_chunk:
                        ci = lo_next[0]
                        t = glopool.tile([128, CHUNKS_PER_CALL, 128], BF16,
                                         tag="glo", name=f"glo{l}_{ci}")
                        c0 = ci * CHUNKS_PER_CALL
                        nc.gpsimd.dma_gather(
                            t[:], z_full[0:SPLIT, :],
                            idxlo_sb[:, c0 * 8:(c0 + CHUNKS_PER_CALL) * 8],
                            num_idxs=CHUNKS_PER_CALL * 128,
                            num_idxs_reg=CHUNKS_PER_CALL * 128,
                            elem_size=128, queue_num=next_q(),
                        )
                        lo_tiles[ci] = t
                        lo_next[0] += 1

                def issue_hi(upto_chunk, l=l, hi_tiles=hi_tiles, hi_next=hi_next,
                             z_full=z_full):
                    while hi_next[0] * CHUNKS_PER_CALL < upto_chunk:
                        ci = hi_next[0]
                        t = ghipool.tile([128, CHUNKS_PER_CALL, 128], BF16,
                                         tag="ghi", name=f"ghi{l}_{ci}")
                        c0 = ci * CHUNKS_PER_CALL
                        nc.gpsimd.dma_gather(
                            t[:], z_full[SPLIT:TOT, :],
                            idxhi_sb[:, c0 * 8:(c0 + CHUNKS_PER_CALL) * 8],
                            num_idxs=CHUNKS_PER_CALL * 128,
                            num_idxs_reg=CHUNKS_PER_CALL * 128,
                            elem_size=128, queue_num=next_q(),
                        )
                        hi_tiles[ci] = t
                        hi_next[0] += 1

                for b in range(NBLK):
                    klo = int(K_lo[b])
                    khi = int(K_hi[b])
                    issue_lo(int(lo_ch_off[b]) + klo)
                    issue_hi(int(hi_ch_off[b]) + khi)
                    ktot = klo + khi
                    ps = psagg.tile([128, 128], F32, tag="aggps",
                                    name=f"agg{l}_{b}")
                    # self-loop contribution from local node-major z
                    nc.tensor.matmul(
                        ps[:], zst[:, b, :], diag_sb[:, b, :],
                        start=True, stop=(ktot == 0))
                    for j in range(ktot):
                        if j < klo:
                            c = int(lo_ch_off[b]) + j
                            msg = lo_tiles[c // CHUNKS_PER_CALL][
                                :, c % CHUNKS_PER_CALL, :]
                            dm, nv = dml_sb, nvl_sb
                        else:
                            c = int(hi_ch_off[b]) + (j - klo)
                            msg = hi_tiles[c // CHUNKS_PER_CALL][
                                :, c % CHUNKS_PER_CALL, :]
                            dm, nv = dmh_sb, nvh_sb
                        oh = ohpool.tile([128, 128], BF16, tag="oh",
                                         name=f"oh{l}_{b}_{j}")
                        nc.vector.tensor_scalar(
                            oh[:], iota_sb[:],
                            dm[:, c:c + 1],
                            nv[:, c:c + 1],
                            mybir.AluOpType.is_equal,
                            mybir.AluOpType.mult,
                        )
                        nc.tensor.matmul(
                            ps[:], msg, oh[:],
                            start=False, stop=(j == ktot - 1))
                    nc.scalar.activation(
                        h_nxt[:, b * 128:(b + 1) * 128], ps[:],
                        mybir.ActivationFunctionType.Relu,
                        bias=bct_sb[:, l:l + 1])
                # drain any padded trailing calls so pools stay consistent
                issue_lo(PCH_lo)
                issue_hi(PCH_hi)
                h_cur = h_nxt

            # ---- pooling + FFN --------------------------------------------
            mx = spool.tile([128, GPC], BF16)
            sm = spool.tile([128, GPC], F32)
            for g in range(GPC):
                nc.vector.tensor_reduce(
                    mx[:, g:g + 1], h_cur[:, g * GSLOT:(g + 1) * GSLOT],
                    mybir.AxisListType.X, mybir.AluOpType.max)
                nc.vector.tensor_reduce(
                    sm[:, g:g + 1], h_cur[:, g * GSLOT:(g + 1) * GSLOT],
                    mybir.AxisListType.X, mybir.AluOpType.add)
            mean = spool.tile([128, GPC], BF16)
            nc.vector.tensor_tensor(
                mean[:], sm[:], invc_sb[:], mybir.AluOpType.mult)

            p1 = psfin.tile([128, GPC], F32, tag="p1")
            nc.tensor.matmul(p1[:], wffn_sb[:, 0, :], mx[:],
                             start=True, stop=False)
            nc.tensor.matmul(p1[:], wffn_sb[:, 1, :], mean[:],
                             start=False, stop=True)
            o1 = spool.tile([128, GPC], BF16)
            nc.scalar.activation(
                o1[:], p1[:], mybir.ActivationFunctionType.Relu,
                bias=bffnt_sb[:, 0:1])

            p2 = psfin.tile([GPC, 2], F32, tag="p2")
            nc.tensor.matmul(p2[:], o1[:], wfin_sb[:], start=True, stop=True)
            osb = spool.tile([GPC, 2], F32)
            nc.vector.tensor_tensor(
                osb[:], p2[:], bfinr_sb[:], mybir.AluOpType.add)
            nc.sync.dma_start(out_d[:], osb[:])

    nc.compile()
    return nc


# ===========================================================================
# entry point
# ===========================================================================
_CACHE = {}


def kernel(x, Wc, bc, W_ffn, b_ffn, W_fin, b_fin, edge_index, batch):
    x = np.ascontiguousarray(np.asarray(x, np.float32))
    Wc = np.ascontiguousarray(np.asarray(Wc, np.float32))
    bc = np.ascontiguousarray(np.asarray(bc, np.float32))
    W_ffn = np.ascontiguousarray(np.asarray(W_ffn, np.float32))
    b_ffn = np.ascontiguousarray(np.asarray(b_ffn, np.float32))
    W_fin = np.ascontiguousarray(np.asarray(W_fin, np.float32))
    b_fin = np.ascontiguousarray(np.asarray(b_fin, np.float32))

    sch = _build_schedule(x, edge_index, batch)

    key = (sch["PCH_lo"], sch["PCH_hi"],
           tuple(sch["K_lo"]), tuple(sch["K_hi"]))
    if key not in _CACHE:
        _CACHE.clear()
        _CACHE[key] = _build_kernel(sch)
    nc = _CACHE[key]

    iota = np.tile(np.arange(128, dtype=np.float32)[None, :], (128, 1))
    ident = np.eye(128, dtype=np.float32)
    bct = bc.T.copy()                       # [128, 3]
    bffnt = b_ffn[:, None].copy()           # [128, 1]
    bfinr = np.tile(b_fin[None, :], (GPC, 1)).astype(np.float32)

    def b16(a):
        return np.ascontiguousarray(a.astype(BF16NP))

    in_maps = []
    for c in range(N_CORES):
        in_maps.append({
            "xfm": b16(sch["x_fm"][c]),
            "wc": b16(Wc), "bct": bct, "wffn": b16(W_ffn), "bffnt": bffnt,
            "wfin": b16(W_fin), "bfinr": bfinr,
            "idxlo": sch["idx_lo"][c], "idxhi": sch["idx_hi"][c],
            "dml": sch["dml"][c], "nvl": sch["nvl"][c],
            "dmh": sch["dmh"][c], "nvh": sch["nvh"][c],
            "diag": b16(sch["diag"][c]),
            "invc": sch["invcnt_rep"][c],
            "iota": b16(iota), "ident": b16(ident),
        })

    _CACHE["in_maps"] = in_maps
    res = run_bass_kernel_spmd(nc, in_maps, core_ids=list(range(N_CORES)))
    out = np.concatenate([res.results[c]["out"] for c in range(N_CORES)], 0)
    return out.astype(np.float32)


def timed_run(inputs=None):
    """Re-run the cached compiled kernel with profiling; returns exec ns."""
    import time
    nc = next(v for k, v in _CACHE.items() if k != "in_maps")
    in_maps = _CACHE["in_maps"]
    # warm re-runs for a wall-clock floor estimate
    walls = []
    for _ in range(3):
        t0 = time.time()
        run_bass_kernel_spmd(nc, in_maps, core_ids=list(range(N_CORES)))
        walls.append(time.time() - t0)
    print(f"warm re-run walls: {[f'{w*1e3:.1f}ms' for w in walls]}")
    try:
        res = run_bass_kernel_spmd(
            nc, in_maps, core_ids=list(range(N_CORES)), trace=True)
        if res.exec_time_ns is not None:
            return res.exec_time_ns
    except Exception as e:
        print(f"(ntff profiling unavailable: {type(e).__name__}; "
              f"reporting warm wall-clock upper bound)")
    return int(min(walls) * 1e9)


if __name__ == "__main__":
    rng = np.random.default_rng(0)
    x = rng.standard_normal((N_NODES, D), dtype=np.float32)
    ei = rng.integers(0, N_NODES, (2, N_EDGES)).astype(np.int64)
    batch = np.sort(rng.integers(0, N_GRAPHS, N_NODES)).astype(np.int64)
    Wc = rng.standard_normal((3, D, D), dtype=np.float32) * 0.05
    out = kernel(x, Wc, np.zeros((3, D), np.float32),
                 rng.standard_normal((2 * D, D), dtype=np.float32) * 0.05,
                 np.zeros((D,), np.float32),
                 rng.standard_normal((D, 2), dtype=np.float32) * 0.05,
                 np.zeros((2,), np.float32), ei, batch)
    print(out.shape, out[:4])


# revision 5
# speedup vs baseline: 1.7849x; 1.0281x over previous
"""DeepWuKong GCN (3-layer GCNConv + max/mean pool + FFN) on 8 TRN2 NeuronCores.

Strategy (graph-level data parallelism, per sharding hint):
  - 128 graphs -> 16 graphs/core; each graph padded to 512 node slots
    (= 4 aligned 128-slot blocks), 8192 node slots/core, 65536 global table
    rows.  Per-layer: each core transforms its own slice (z = h @ Wc[l],
    feature-major on chip, bf16), transposes to node-major, AllGathers the
    full bf16 z table into shared HBM, then processes the edges whose dst it
    owns: dma_gather (SWDGE row gather, 256B bf16 rows) pulls z[src] rows in
    uniform 1024-index calls, a norm-weighted one-hot (built on DVE from
    iota==dstmod, bf16) maps each 128-edge chunk into its dst block via a PE
    matmul accumulated in fp32 PSUM, and ScalarE applies bias+ReLU into the
    next bf16 feature-major h tile.
  - Self-loops (the PyG-added one plus any data self-edges) never touch the
    gather path: they are one local matmul per dst block against a constant
    diagonal weight matrix read from the node-major z kept in SBUF.
  - Segment pooling is per-core local (graph slots are contiguous), FFN is
    two tiny matmuls; host stitches the 8 [16,2] outputs.

Everything on the message path is bf16 (z table, gathers, one-hots,
matmul operands); accumulation stays fp32 in PSUM.  Gather calls are
decoupled from dst-block boundaries: each SWDGE call covers 8 chunks
(1024 indices, the per-call limit) to amortize the ~1us fixed Q7 cost.
"""
import sys

sys.path.insert(0, "/opt/trn_rl_repo")

import numpy as np
import ml_dtypes

import concourse.bacc as bacc
import concourse.bass as bass
import concourse.mybir as mybir
import concourse.tile as tile
from concourse.bass_utils import run_bass_kernel_spmd

BF16NP = np.dtype(ml_dtypes.bfloat16)

# ---- problem constants (hardcoded per spec) --------------------------------
N_NODES = 50000
N_EDGES = 600000
N_GRAPHS = 128
D = 128
N_LAYERS = 3
N_CORES = 8
GPC = N_GRAPHS // N_CORES      # 16 graphs per core
GSLOT = 512                    # node slots per graph (4 blocks of 128)
NLOC = GPC * GSLOT             # 8192 node slots per core
NBLK = NLOC // 128             # 64 blocks per core
TOT = N_CORES * NLOC           # 65536 table rows
SPLIT = 32768                  # int16 gather index split
BPG = GSLOT // 128             # blocks per graph

F32 = mybir.dt.float32
BF16 = mybir.dt.bfloat16
I16 = mybir.dt.int16

# SWDGE tuning: a single dma_gather is limited to ~1024 indices (Q7-local
# idx scratch; exceeding it crashes the device).
DMA_SCRATCH = 16384
CHUNKS_PER_CALL = 8            # 8 chunks x 128 idx = 1024 idx per call
N_QUEUES = 4


# ===========================================================================
# host-side schedule construction
# ===========================================================================
def _build_schedule(x, edge_index, batch):
    x = np.asarray(x, np.float32)
    ei = np.asarray(edge_index).astype(np.int64)
    batch = np.asarray(batch).astype(np.int64)

    counts = np.bincount(batch, minlength=N_GRAPHS)
    assert counts.max() <= GSLOT, f"graph too big: {counts.max()}"

    deg = np.bincount(ei[1], minlength=N_NODES).astype(np.float64) + 1.0
    dis = 1.0 / np.sqrt(deg)

    graph_start = np.zeros(N_GRAPHS + 1, np.int64)
    np.cumsum(counts, out=graph_start[1:])

    # degree-balanced placement of each graph's nodes into its BPG blocks
    newidx = np.full(N_NODES, -1, np.int64)
    for g in range(N_GRAPHS):
        nodes = np.arange(graph_start[g], graph_start[g + 1])
        if len(nodes) == 0:
            continue
        order = np.argsort(-deg[nodes], kind="stable")
        base = (g // GPC) * NLOC + (g % GPC) * GSLOT
        bin_load = np.zeros(BPG)
        bin_fill = np.zeros(BPG, np.int64)
        for n in nodes[order]:
            cand = np.argsort(bin_load, kind="stable")
            for b in cand:
                if bin_fill[b] < 128:
                    break
            newidx[n] = base + b * 128 + bin_fill[b]
            bin_fill[b] += 1
            bin_load[b] += deg[n]
    assert (newidx[batch >= 0] >= 0).all()

    # self weights: the PyG-added loop plus any data self-edges, all with
    # weight dis[n]^2; these go through the local diag matmul, not gathers
    selfmask = ei[0] == ei[1]
    nself = np.bincount(ei[1][selfmask], minlength=N_NODES)
    wself = (1.0 + nself) * dis * dis

    diag = np.zeros((N_CORES, 128, NBLK, 128), np.float32)
    pall = newidx
    cc = pall // NLOC
    bb = (pall % NLOC) // 128
    ss = pall % 128
    diag[cc, ss, bb, ss] = wself.astype(np.float32)

    # non-self edge list, keyed by owner core of dst
    src = ei[0][~selfmask]
    dst = ei[1][~selfmask]
    w = (dis[src] * dis[dst]).astype(np.float32)
    psrc = newidx[src]
    pdst = newidx[dst]
    core = pdst // NLOC
    ldst = pdst % NLOC
    blk = ldst // 128
    hi = (psrc >= SPLIT).astype(np.int64)

    cnt = np.zeros((N_CORES, NBLK, 2), np.int64)
    np.add.at(cnt, (core, blk, hi), 1)
    need = -(-cnt // 128)
    K = need.max(axis=0)                       # [NBLK, 2], same on all cores
    K_lo = K[:, 0].astype(int)
    K_hi = K[:, 1].astype(int)
    NCH_lo = int(K_lo.sum())
    NCH_hi = int(K_hi.sum())
    # pad chunk counts to a multiple of CHUNKS_PER_CALL for uniform calls
    PCH_lo = -(-NCH_lo // CHUNKS_PER_CALL) * CHUNKS_PER_CALL
    PCH_hi = -(-NCH_hi // CHUNKS_PER_CALL) * CHUNKS_PER_CALL

    lo_ch_off = np.zeros(NBLK + 1, np.int64)
    np.cumsum(K_lo, out=lo_ch_off[1:])
    hi_ch_off = np.zeros(NBLK + 1, np.int64)
    np.cumsum(K_hi, out=hi_ch_off[1:])

    idx_lo = np.zeros((N_CORES, PCH_lo * 128), np.int16)
    idx_hi = np.zeros((N_CORES, PCH_hi * 128), np.int16)
    dml = np.full((N_CORES, 128, PCH_lo), -1.0, np.float32)
    nvl = np.zeros((N_CORES, 128, PCH_lo), np.float32)
    dmh = np.full((N_CORES, 128, PCH_hi), -1.0, np.float32)
    nvh = np.zeros((N_CORES, 128, PCH_hi), np.float32)

    # vectorized per-(core,blk,bucket) slot assignment
    sort = np.lexsort((hi, blk, core))
    s_core, s_blk, s_hi = core[sort], blk[sort], hi[sort]
    s_ps, s_ld, s_w = psrc[sort], ldst[sort], w[sort]
    gid = (s_core * NBLK + s_blk) * 2 + s_hi
    first = np.ones(len(gid), bool)
    first[1:] = gid[1:] != gid[:-1]
    gstart = np.zeros(len(gid), np.int64)
    idxs_first = np.flatnonzero(first)
    gstart[idxs_first] = idxs_first
    gstart = np.maximum.accumulate(gstart)
    pos = np.arange(len(gid)) - gstart                  # within-group position

    chcol = np.where(s_hi == 0, lo_ch_off[s_blk], hi_ch_off[s_blk]) \
        + pos // 128
    slot = chcol * 128 + pos % 128
    part = pos % 128
    val = np.where(s_hi == 0, s_ps, s_ps - SPLIT).astype(np.int16)
    lom = s_hi == 0
    idx_lo[s_core[lom], slot[lom]] = val[lom]
    idx_hi[s_core[~lom], slot[~lom]] = val[~lom]
    dml[s_core[lom], part[lom], chcol[lom]] = (s_ld[lom] % 128).astype(np.float32)
    nvl[s_core[lom], part[lom], chcol[lom]] = s_w[lom]
    dmh[s_core[~lom], part[~lom], chcol[~lom]] = (s_ld[~lom] % 128).astype(np.float32)
    nvh[s_core[~lom], part[~lom], chcol[~lom]] = s_w[~lom]

    def wrap_idx(a):                 # [slots] -> [128, slots/16], 8x replicated
        w16 = a.reshape(-1, 16).T
        return np.tile(w16, (8, 1)).copy()

    idx_lo_w = np.stack([wrap_idx(idx_lo[c]) for c in range(N_CORES)])
    idx_hi_w = np.stack([wrap_idx(idx_hi[c]) for c in range(N_CORES)])

    xpad = np.zeros((TOT, D), np.float32)
    xpad[newidx] = x
    x_fm = np.stack([xpad[c * NLOC:(c + 1) * NLOC].T.copy()
                     for c in range(N_CORES)])

    invcnt = (1.0 / np.maximum(counts, 1)).astype(np.float32)
    invcnt_rep = np.stack([
        np.tile(invcnt[c * GPC:(c + 1) * GPC], (128, 1)) for c in range(N_CORES)
    ]).astype(np.float32)

    return dict(
        K_lo=K_lo, K_hi=K_hi,
        NCH_lo=NCH_lo, NCH_hi=NCH_hi, PCH_lo=PCH_lo, PCH_hi=PCH_hi,
        lo_ch_off=lo_ch_off, hi_ch_off=hi_ch_off,
        idx_lo=idx_lo_w, idx_hi=idx_hi_w,
        dml=dml, nvl=nvl, dmh=dmh, nvh=nvh, diag=diag,
        x_fm=x_fm, invcnt_rep=invcnt_rep,
    )


# ===========================================================================
# device kernel
# ===========================================================================
def _build_kernel(sch):
    K_lo, K_hi = sch["K_lo"], sch["K_hi"]
    lo_ch_off, hi_ch_off = sch["lo_ch_off"], sch["hi_ch_off"]
    PCH_lo, PCH_hi = sch["PCH_lo"], sch["PCH_hi"]
    NCALL_lo = PCH_lo // CHUNKS_PER_CALL
    NCALL_hi = PCH_hi // CHUNKS_PER_CALL

    nc = bacc.Bacc(
        "TRN2",
        target_bir_lowering=False,
        debug=False,
        num_devices=N_CORES,
        num_swdge_queues=N_QUEUES,
        dynamic_dma_scratch_size=DMA_SCRATCH,
    )

    xfm_d = nc.dram_tensor("xfm", [128, NLOC], BF16, kind="ExternalInput")
    wc_d = nc.dram_tensor("wc", [N_LAYERS, 128, 128], BF16, kind="ExternalInput")
    bct_d = nc.dram_tensor("bct", [128, N_LAYERS], F32, kind="ExternalInput")
    wffn_d = nc.dram_tensor("wffn", [256, 128], BF16, kind="ExternalInput")
    bffnt_d = nc.dram_tensor("bffnt", [128, 1], F32, kind="ExternalInput")
    wfin_d = nc.dram_tensor("wfin", [128, 2], BF16, kind="ExternalInput")
    bfinr_d = nc.dram_tensor("bfinr", [GPC, 2], F32, kind="ExternalInput")
    idxlo_d = nc.dram_tensor("idxlo", [128, PCH_lo * 8], I16, kind="ExternalInput")
    idxhi_d = nc.dram_tensor("idxhi", [128, PCH_hi * 8], I16, kind="ExternalInput")
    dml_d = nc.dram_tensor("dml", [128, PCH_lo], F32, kind="ExternalInput")
    nvl_d = nc.dram_tensor("nvl", [128, PCH_lo], F32, kind="ExternalInput")
    dmh_d = nc.dram_tensor("dmh", [128, PCH_hi], F32, kind="ExternalInput")
    nvh_d = nc.dram_tensor("nvh", [128, PCH_hi], F32, kind="ExternalInput")
    diag_d = nc.dram_tensor("diag", [128, NBLK, 128], BF16, kind="ExternalInput")
    invc_d = nc.dram_tensor("invc", [128, GPC], F32, kind="ExternalInput")
    iota_d = nc.dram_tensor("iota", [128, 128], BF16, kind="ExternalInput")
    ident_d = nc.dram_tensor("ident", [128, 128], BF16, kind="ExternalInput")
    out_d = nc.dram_tensor("out", [GPC, 2], F32, kind="ExternalOutput")

    RG = [list(range(N_CORES))]

    with tile.TileContext(nc) as tc:
        with (
            tc.tile_pool(name="consts", bufs=1) as consts,
            tc.tile_pool(name="hpool", bufs=2) as hpool,
            tc.tile_pool(name="zpool", bufs=2) as zpool,
            tc.tile_pool(name="zstpool", bufs=2) as zstpool,
            tc.tile_pool(name="glopool", bufs=8) as glopool,
            tc.tile_pool(name="ghipool", bufs=8) as ghipool,
            tc.tile_pool(name="ohpool", bufs=8) as ohpool,
            tc.tile_pool(name="spool", bufs=1) as spool,
            tc.tile_pool(name="ps512", bufs=2, space="PSUM") as ps512,
            tc.tile_pool(name="ps128", bufs=2, space="PSUM") as ps128,
            tc.tile_pool(name="psagg", bufs=2, space="PSUM") as psagg,
            tc.tile_pool(name="psfin", bufs=1, space="PSUM") as psfin,
            tc.tile_pool(name="dram", bufs=1, space="DRAM") as dram,
        ):
            # ---- load constants -------------------------------------------
            wc_sb = consts.tile([128, N_LAYERS, 128], BF16)
            nc.sync.dma_start(wc_sb[:], wc_d[:].rearrange("l p f -> p l f"))
            bct_sb = consts.tile([128, N_LAYERS], F32)
            nc.sync.dma_start(bct_sb[:], bct_d[:])
            wffn_sb = consts.tile([128, 2, 128], BF16)
            nc.sync.dma_start(
                wffn_sb[:], wffn_d[:].rearrange("(h p) f -> p h f", p=128))
            bffnt_sb = consts.tile([128, 1], F32)
            nc.sync.dma_start(bffnt_sb[:], bffnt_d[:])
            wfin_sb = consts.tile([128, 2], BF16)
            nc.sync.dma_start(wfin_sb[:], wfin_d[:])
            bfinr_sb = consts.tile([GPC, 2], F32)
            nc.sync.dma_start(bfinr_sb[:], bfinr_d[:])
            idxlo_sb = consts.tile([128, PCH_lo * 8], I16)
            nc.sync.dma_start(idxlo_sb[:], idxlo_d[:])
            idxhi_sb = consts.tile([128, PCH_hi * 8], I16)
            nc.sync.dma_start(idxhi_sb[:], idxhi_d[:])
            dml_sb = consts.tile([128, PCH_lo], F32)
            nc.sync.dma_start(dml_sb[:], dml_d[:])
            nvl_sb = consts.tile([128, PCH_lo], F32)
            nc.sync.dma_start(nvl_sb[:], nvl_d[:])
            dmh_sb = consts.tile([128, PCH_hi], F32)
            nc.sync.dma_start(dmh_sb[:], dmh_d[:])
            nvh_sb = consts.tile([128, PCH_hi], F32)
            nc.sync.dma_start(nvh_sb[:], nvh_d[:])
            diag_sb = consts.tile([128, NBLK, 128], BF16)
            nc.sync.dma_start(diag_sb[:], diag_d[:])
            invc_sb = consts.tile([128, GPC], F32)
            nc.sync.dma_start(invc_sb[:], invc_d[:])
            iota_sb = consts.tile([128, 128], BF16)
            nc.sync.dma_start(iota_sb[:], iota_d[:])
            ident_sb = consts.tile([128, 128], BF16)
            nc.sync.dma_start(ident_sb[:], ident_d[:])

            h_cur = hpool.tile([128, NLOC], BF16, tag="h", name="h_init")
            nc.sync.dma_start(h_cur[:], xfm_d[:])

            qctr = [0]

            def next_q():
                q = qctr[0] % N_QUEUES
                qctr[0] += 1
                return q

            for l in range(N_LAYERS):
                # ---- transform own slice: z[fo, n] = sum_fi Wc[fi,fo] h[fi,n]
                zst = zstpool.tile([128, NBLK, 128], BF16, tag="zst",
                                   name=f"zst{l}")
                for g in range(GPC):
                    zps = ps512.tile([128, 512], F32, tag="zps",
                                     name=f"zps{l}_{g}")
                    nc.tensor.matmul(
                        zps[:], wc_sb[:, l, :],
                        h_cur[:, g * 512:(g + 1) * 512],
                        start=True, stop=True)
                    zsb = zpool.tile([128, 512], BF16, tag="zsb",
                                     name=f"zsb{l}_{g}")
                    nc.scalar.activation(
                        zsb[:], zps[:], mybir.ActivationFunctionType.Copy)
                    for t in range(4):
                        tps = ps128.tile([128, 128], BF16, tag="tps",
                                         name=f"tps{l}_{g}_{t}")
                        nc.tensor.transpose(
                            tps[:], zsb[:, t * 128:(t + 1) * 128], ident_sb[:])
                        nc.scalar.activation(
                            zst[:, g * 4 + t, :], tps[:],
                            mybir.ActivationFunctionType.Copy)
                z_own = dram.tile([NLOC, 128], BF16, tag="zown", bufs=2,
                                  name=f"zown{l}")
                nc.sync.dma_start(
                    z_own[:].rearrange("(b p) f -> p b f", p=128), zst[:])

                # ---- AllGather the z table --------------------------------
                z_full = dram.tile([TOT, 128], BF16, tag="zfull", bufs=2,
                                   addr_space="Shared", name=f"zfull{l}")
                nc.gpsimd.collective_compute(
                    "AllGather", mybir.AluOpType.bypass,
                    replica_groups=RG,
                    ins=[z_own[:].opt()],
                    outs=[z_full[:].opt()],
                )

                # ---- gather + aggregate -----------------------------------
                h_nxt = hpool.tile([128, NLOC], BF16, tag="h", name=f"h{l + 1}")
                lo_tiles = {}
                hi_tiles = {}
                lo_next = [0]
                hi_next = [0]

                def issue_lo(upto_chunk, l=l, lo_tiles=lo_tiles, lo_next=lo_next,
                             z_full=z_full):
                    while lo_next[0] * CHUNKS_PER_CALL < upto_chunk:
                        ci = lo_next[0]
                        t = glopool.tile([128, CHUNKS_PER_CALL, 128], BF16,
                                         tag="glo", name=f"glo{l}_{ci}")
                        c0 = ci * CHUNKS_PER_CALL
                        nc.gpsimd.dma_gather(
                            t[:], z_full[0:SPLIT, :],
                            idxlo_sb[:, c0 * 8:(c0 + CHUNKS_PER_CALL) * 8],
                            num_idxs=CHUNKS_PER_CALL * 128,
                            num_idxs_reg=CHUNKS_PER_CALL * 128,
                            elem_size=128, queue_num=next_q(),
                        )
                        lo_tiles[ci] = t
                        lo_next[0] += 1

                def issue_hi(upto_chunk, l=l, hi_tiles=hi_tiles, hi_next=hi_next,
                             z_full=z_full):
                    while hi_next[0] * CHUNKS_PER_CALL < upto_chunk:
                        ci = hi_next[0]
                        t = ghipool.tile([128, CHUNKS_PER_CALL, 128], BF16,
                                         tag="ghi", name=f"ghi{l}_{ci}")
                        c0 = ci * CHUNKS_PER_CALL
                        nc.gpsimd.dma_gather(
                            t[:], z_full[SPLIT:TOT, :],
                            idxhi_sb[:, c0 * 8:(c0 + CHUNKS_PER_CALL) * 8],
                            num_idxs=CHUNKS_PER_CALL * 128,
                            num_idxs_reg=CHUNKS_PER_CALL * 128,
                            elem_size=128, queue_num=next_q(),
                        )
                        hi_tiles[ci] = t
                        hi_next[0] += 1

                # interleave all lo/hi gather calls upfront: the in-order
                # gpsimd engine plus pool WAR semaphores self-pace them ~bufs
                # calls ahead of matmul consumption
                for ci in range(max(NCALL_lo, NCALL_hi)):
                    issue_lo(min((ci + 1) * CHUNKS_PER_CALL, PCH_lo))
                    issue_hi(min((ci + 1) * CHUNKS_PER_CALL, PCH_hi))
                for b in range(NBLK):
                    klo = int(K_lo[b])
                    khi = int(K_hi[b])
                    ktot = klo + khi
                    ps = psagg.tile([128, 128], F32, tag="aggps",
                                    name=f"agg{l}_{b}")
                    # self-loop contribution from local node-major z
                    nc.tensor.matmul(
                        ps[:], zst[:, b, :], diag_sb[:, b, :],
                        start=True, stop=(ktot == 0))
                    for j in range(ktot):
                        if j < klo:
                            c = int(lo_ch_off[b]) + j
                            msg = lo_tiles[c // CHUNKS_PER_CALL][
                                :, c % CHUNKS_PER_CALL, :]
                            dm, nv = dml_sb, nvl_sb
                        else:
                            c = int(hi_ch_off[b]) + (j - klo)
                            msg = hi_tiles[c // CHUNKS_PER_CALL][
                                :, c % CHUNKS_PER_CALL, :]
                            dm, nv = dmh_sb, nvh_sb
                        oh = ohpool.tile([128, 128], BF16, tag="oh",
                                         name=f"oh{l}_{b}_{j}")
                        nc.vector.tensor_scalar(
                            oh[:], iota_sb[:],
                            dm[:, c:c + 1],
                            nv[:, c:c + 1],
                            mybir.AluOpType.is_equal,
                            mybir.AluOpType.mult,
                        )
                        nc.tensor.matmul(
                            ps[:], msg, oh[:],
                            start=False, stop=(j == ktot - 1))
                    nc.scalar.activation(
                        h_nxt[:, b * 128:(b + 1) * 128], ps[:],
                        mybir.ActivationFunctionType.Relu,
                        bias=bct_sb[:, l:l + 1])
                h_cur = h_nxt

            # ---- pooling + FFN --------------------------------------------
            mx = spool.tile([128, GPC], BF16)
            sm = spool.tile([128, GPC], F32)
            for g in range(GPC):
                nc.vector.tensor_reduce(
                    mx[:, g:g + 1], h_cur[:, g * GSLOT:(g + 1) * GSLOT],
                    mybir.AxisListType.X, mybir.AluOpType.max)
                nc.vector.tensor_reduce(
                    sm[:, g:g + 1], h_cur[:, g * GSLOT:(g + 1) * GSLOT],
                    mybir.AxisListType.X, mybir.AluOpType.add)
            mean = spool.tile([128, GPC], BF16)
            nc.vector.tensor_tensor(
                mean[:], sm[:], invc_sb[:], mybir.AluOpType.mult)

            p1 = psfin.tile([128, GPC], F32, tag="p1")
            nc.tensor.matmul(p1[:], wffn_sb[:, 0, :], mx[:],
                             start=True, stop=False)
            nc.tensor.matmul(p1[:], wffn_sb[:, 1, :], mean[:],
                             start=False, stop=True)
            o1 = spool.tile([128, GPC], BF16)
            nc.scalar.activation(
                o1[:], p1[:], mybir.ActivationFunctionType.Relu,
                bias=bffnt_sb[:, 0:1])

            p2 = psfin.tile([GPC, 2], F32, tag="p2")
            nc.tensor.matmul(p2[:], o1[:], wfin_sb[:], start=True, stop=True)
            osb = spool.tile([GPC, 2], F32)
            nc.vector.tensor_tensor(
                osb[:], p2[:], bfinr_sb[:], mybir.AluOpType.add)
            nc.sync.dma_start(out_d[:], osb[:])

    nc.compile()
    return nc


# ===========================================================================
# entry point
# ===========================================================================
_CACHE = {}


def kernel(x, Wc, bc, W_ffn, b_ffn, W_fin, b_fin, edge_index, batch):
    x = np.ascontiguousarray(np.asarray(x, np.float32))
    Wc = np.ascontiguousarray(np.asarray(Wc, np.float32))
    bc = np.ascontiguousarray(np.asarray(bc, np.float32))
    W_ffn = np.ascontiguousarray(np.asarray(W_ffn, np.float32))
    b_ffn = np.ascontiguousarray(np.asarray(b_ffn, np.float32))
    W_fin = np.ascontiguousarray(np.asarray(W_fin, np.float32))
    b_fin = np.ascontiguousarray(np.asarray(b_fin, np.float32))

    sch = _build_schedule(x, edge_index, batch)

    key = (sch["PCH_lo"], sch["PCH_hi"],
           tuple(sch["K_lo"]), tuple(sch["K_hi"]))
    if key not in _CACHE:
        _CACHE.clear()
        _CACHE[key] = _build_kernel(sch)
    nc = _CACHE[key]

    iota = np.tile(np.arange(128, dtype=np.float32)[None, :], (128, 1))
    ident = np.eye(128, dtype=np.float32)
    bct = bc.T.copy()                       # [128, 3]
    bffnt = b_ffn[:, None].copy()           # [128, 1]
    bfinr = np.tile(b_fin[None, :], (GPC, 1)).astype(np.float32)

    def b16(a):
        return np.ascontiguousarray(a.astype(BF16NP))

    in_maps = []
    for c in range(N_CORES):
        in_maps.append({
            "xfm": b16(sch["x_fm"][c]),
            "wc": b16(Wc), "bct": bct, "wffn": b16(W_ffn), "bffnt": bffnt,
            "wfin": b16(W_fin), "bfinr": bfinr,
            "idxlo": sch["idx_lo"][c], "idxhi": sch["idx_hi"][c],
            "dml": sch["dml"][c], "nvl": sch["nvl"][c],
            "dmh": sch["dmh"][c], "nvh": sch["nvh"][c],
            "diag": b16(sch["diag"][c]),
            "invc": sch["invcnt_rep"][c],
            "iota": b16(iota), "ident": b16(ident),
        })

    _CACHE["in_maps"] = in_maps
    res = run_bass_kernel_spmd(nc, in_maps, core_ids=list(range(N_CORES)))
    out = np.concatenate([res.results[c]["out"] for c in range(N_CORES)], 0)
    return out.astype(np.float32)


def timed_run(inputs=None):
    """Re-run the cached compiled kernel with profiling; returns exec ns."""
    import time
    nc = next(v for k, v in _CACHE.items() if k != "in_maps")
    in_maps = _CACHE["in_maps"]
    # warm re-runs for a wall-clock floor estimate
    walls = []
    for _ in range(3):
        t0 = time.time()
        run_bass_kernel_spmd(nc, in_maps, core_ids=list(range(N_CORES)))
        walls.append(time.time() - t0)
    print(f"warm re-run walls: {[f'{w*1e3:.1f}ms' for w in walls]}")
    try:
        res = run_bass_kernel_spmd(
            nc, in_maps, core_ids=list(range(N_CORES)), trace=True)
        if res.exec_time_ns is not None:
            return res.exec_time_ns
    except Exception as e:
        print(f"(ntff profiling unavailable: {type(e).__name__}; "
              f"reporting warm wall-clock upper bound)")
    return int(min(walls) * 1e9)


if __name__ == "__main__":
    rng = np.random.default_rng(0)
    x = rng.standard_normal((N_NODES, D), dtype=np.float32)
    ei = rng.integers(0, N_NODES, (2, N_EDGES)).astype(np.int64)
    batch = np.sort(rng.integers(0, N_GRAPHS, N_NODES)).astype(np.int64)
    Wc = rng.standard_normal((3, D, D), dtype=np.float32) * 0.05
    out = kernel(x, Wc, np.zeros((3, D), np.float32),
                 rng.standard_normal((2 * D, D), dtype=np.float32) * 0.05,
                 np.zeros((D,), np.float32),
                 rng.standard_normal((D, 2), dtype=np.float32) * 0.05,
                 np.zeros((2,), np.float32), ei, batch)
    print(out.shape, out[:4])


# revision 7
# speedup vs baseline: 1.8005x; 1.0087x over previous
"""DeepWuKong GCN (3-layer GCNConv + max/mean pool + FFN) on 8 TRN2 NeuronCores.

Strategy (graph-level data parallelism, per sharding hint):
  - 128 graphs -> 16 graphs/core; each graph padded to 512 node slots
    (= 4 aligned 128-slot blocks), 8192 node slots/core, 65536 global table
    rows.  Per-layer: each core transforms its own slice (z = h @ Wc[l],
    feature-major on chip, bf16), transposes to node-major, AllGathers the
    full bf16 z table into shared HBM, then processes the edges whose dst it
    owns: dma_gather (SWDGE row gather, 256B bf16 rows) pulls z[src] rows in
    uniform 1024-index calls, and a PE matmul against a host-precomputed
    norm-weighted one-hot (streamed from DRAM in 64-chunk batches) maps each
    128-edge chunk into its dst block, accumulated in fp32 PSUM; ScalarE
    applies bias+ReLU into the next bf16 feature-major h tile.
  - Gathers use prepare_only + trigger_dma so the GpSimd engine only holds
    the shared SBUF port for descriptor generation (~1us), not for the DMA
    drain; calls on 4 SWDGE queues overlap deeply.
  - Self-loops (the PyG-added loop plus any data self-edges) never touch the
    gather path: they are one local matmul per dst block against a constant
    diagonal weight matrix read from the node-major z kept in SBUF.
  - Segment pooling is per-core local (graph slots are contiguous), FFN is
    two tiny matmuls; host stitches the 8 [16,2] outputs.

Everything on the message path is bf16 (z table, gathers, one-hots,
matmul operands); accumulation stays fp32 in PSUM.
"""
import sys

sys.path.insert(0, "/opt/trn_rl_repo")

import numpy as np
import ml_dtypes

import concourse.bacc as bacc
import concourse.bass as bass
import concourse.mybir as mybir
import concourse.tile as tile
from concourse.bass_utils import run_bass_kernel_spmd

BF16NP = np.dtype(ml_dtypes.bfloat16)

# ---- problem constants (hardcoded per spec) --------------------------------
N_NODES = 50000
N_EDGES = 600000
N_GRAPHS = 128
D = 128
N_LAYERS = 3
N_CORES = 8
GPC = N_GRAPHS // N_CORES      # 16 graphs per core
GSLOT = 512                    # node slots per graph (4 blocks of 128)
NLOC = GPC * GSLOT             # 8192 node slots per core
NBLK = NLOC // 128             # 64 blocks per core
TOT = N_CORES * NLOC           # 65536 table rows
SPLIT = 32768                  # int16 gather index split
BPG = GSLOT // 128             # blocks per graph

F32 = mybir.dt.float32
BF16 = mybir.dt.bfloat16
I16 = mybir.dt.int16

# SWDGE tuning: a single dma_gather is limited to ~1024 indices (Q7-local
# idx scratch; exceeding it crashes the device).
DMA_SCRATCH = 16384
CHUNKS_PER_CALL = 8            # 8 chunks x 128 idx = 1024 idx per call
N_QUEUES = 4
OH_BATCH = 64                  # one-hot chunks per DMA batch (2MB bf16)


# ===========================================================================
# host-side schedule construction
# ===========================================================================
def _build_schedule(x, edge_index, batch):
    x = np.asarray(x, np.float32)
    ei = np.asarray(edge_index).astype(np.int64)
    batch = np.asarray(batch).astype(np.int64)

    counts = np.bincount(batch, minlength=N_GRAPHS)
    assert counts.max() <= GSLOT, f"graph too big: {counts.max()}"

    deg = np.bincount(ei[1], minlength=N_NODES).astype(np.float64) + 1.0
    dis = 1.0 / np.sqrt(deg)

    graph_start = np.zeros(N_GRAPHS + 1, np.int64)
    np.cumsum(counts, out=graph_start[1:])

    # degree-balanced placement of each graph's nodes into its BPG blocks
    newidx = np.full(N_NODES, -1, np.int64)
    for g in range(N_GRAPHS):
        nodes = np.arange(graph_start[g], graph_start[g + 1])
        if len(nodes) == 0:
            continue
        order = np.argsort(-deg[nodes], kind="stable")
        base = (g // GPC) * NLOC + (g % GPC) * GSLOT
        bin_load = np.zeros(BPG)
        bin_fill = np.zeros(BPG, np.int64)
        for n in nodes[order]:
            cand = np.argsort(bin_load, kind="stable")
            for b in cand:
                if bin_fill[b] < 128:
                    break
            newidx[n] = base + b * 128 + bin_fill[b]
            bin_fill[b] += 1
            bin_load[b] += deg[n]
    assert (newidx[batch >= 0] >= 0).all()

    # self weights: the PyG-added loop plus any data self-edges, all with
    # weight dis[n]^2; these go through the local diag matmul, not gathers
    selfmask = ei[0] == ei[1]
    nself = np.bincount(ei[1][selfmask], minlength=N_NODES)
    wself = (1.0 + nself) * dis * dis

    diag = np.zeros((N_CORES, 128, NBLK, 128), np.float32)
    pall = newidx
    cc = pall // NLOC
    bb = (pall % NLOC) // 128
    ss = pall % 128
    diag[cc, ss, bb, ss] = wself.astype(np.float32)

    # non-self edge list, keyed by owner core of dst
    src = ei[0][~selfmask]
    dst = ei[1][~selfmask]
    w = (dis[src] * dis[dst]).astype(np.float32)
    psrc = newidx[src]
    pdst = newidx[dst]
    core = pdst // NLOC
    ldst = pdst % NLOC
    blk = ldst // 128
    hi = (psrc >= SPLIT).astype(np.int64)

    cnt = np.zeros((N_CORES, NBLK, 2), np.int64)
    np.add.at(cnt, (core, blk, hi), 1)
    need = -(-cnt // 128)
    K = need.max(axis=0)                       # [NBLK, 2], same on all cores
    K_lo = K[:, 0].astype(int)
    K_hi = K[:, 1].astype(int)
    NCH_lo = int(K_lo.sum())
    NCH_hi = int(K_hi.sum())
    # pad chunk counts to a multiple of CHUNKS_PER_CALL for uniform calls
    PCH_lo = -(-NCH_lo // CHUNKS_PER_CALL) * CHUNKS_PER_CALL
    PCH_hi = -(-NCH_hi // CHUNKS_PER_CALL) * CHUNKS_PER_CALL

    lo_ch_off = np.zeros(NBLK + 1, np.int64)
    np.cumsum(K_lo, out=lo_ch_off[1:])
    hi_ch_off = np.zeros(NBLK + 1, np.int64)
    np.cumsum(K_hi, out=hi_ch_off[1:])

    idx_lo = np.zeros((N_CORES, PCH_lo * 128), np.int16)
    idx_hi = np.zeros((N_CORES, PCH_hi * 128), np.int16)
    dml = np.full((N_CORES, 128, PCH_lo), -1.0, np.float32)
    nvl = np.zeros((N_CORES, 128, PCH_lo), np.float32)
    dmh = np.full((N_CORES, 128, PCH_hi), -1.0, np.float32)
    nvh = np.zeros((N_CORES, 128, PCH_hi), np.float32)

    # vectorized per-(core,blk,bucket) slot assignment
    sort = np.lexsort((hi, blk, core))
    s_core, s_blk, s_hi = core[sort], blk[sort], hi[sort]
    s_ps, s_ld, s_w = psrc[sort], ldst[sort], w[sort]
    gid = (s_core * NBLK + s_blk) * 2 + s_hi
    first = np.ones(len(gid), bool)
    first[1:] = gid[1:] != gid[:-1]
    gstart = np.zeros(len(gid), np.int64)
    idxs_first = np.flatnonzero(first)
    gstart[idxs_first] = idxs_first
    gstart = np.maximum.accumulate(gstart)
    pos = np.arange(len(gid)) - gstart                  # within-group position

    chcol = np.where(s_hi == 0, lo_ch_off[s_blk], hi_ch_off[s_blk]) \
        + pos // 128
    slot = chcol * 128 + pos % 128
    part = pos % 128
    val = np.where(s_hi == 0, s_ps, s_ps - SPLIT).astype(np.int16)
    lom = s_hi == 0
    idx_lo[s_core[lom], slot[lom]] = val[lom]
    idx_hi[s_core[~lom], slot[~lom]] = val[~lom]
    dml[s_core[lom], part[lom], chcol[lom]] = (s_ld[lom] % 128).astype(np.float32)
    nvl[s_core[lom], part[lom], chcol[lom]] = s_w[lom]
    dmh[s_core[~lom], part[~lom], chcol[~lom]] = (s_ld[~lom] % 128).astype(np.float32)
    nvh[s_core[~lom], part[~lom], chcol[~lom]] = s_w[~lom]

    # ---- precomputed one-hot tables in consumption order ------------------
    # consumption position p -> (bucket, chunk); order: per block, lo chunks
    # then hi chunks
    ord_bucket = []
    ord_chunk = []
    for b in range(NBLK):
        for j in range(K_lo[b]):
            ord_bucket.append(0)
            ord_chunk.append(int(lo_ch_off[b]) + j)
        for j in range(K_hi[b]):
            ord_bucket.append(1)
            ord_chunk.append(int(hi_ch_off[b]) + j)
    NCHT = len(ord_bucket)
    NCHT_pad = -(-NCHT // OH_BATCH) * OH_BATCH
    ob = np.array(ord_bucket)
    oc = np.array(ord_chunk)

    iota = np.arange(128, dtype=np.float32)
    oh_all = np.zeros((N_CORES, 128, NCHT_pad, 128), BF16NP)
    for c in range(N_CORES):
        dm_ord = np.where(ob[None, :] == 0, dml[c][:, oc], dmh[c][:, oc])
        nv_ord = np.where(ob[None, :] == 0, nvl[c][:, oc], nvh[c][:, oc])
        oh = (dm_ord[:, :, None] == iota[None, None, :]) * nv_ord[:, :, None]
        oh_all[c, :, :NCHT, :] = oh.astype(BF16NP)

    def wrap_idx(a):                 # [slots] -> [128, slots/16], 8x replicated
        w16 = a.reshape(-1, 16).T
        return np.tile(w16, (8, 1)).copy()

    idx_lo_w = np.stack([wrap_idx(idx_lo[c]) for c in range(N_CORES)])
    idx_hi_w = np.stack([wrap_idx(idx_hi[c]) for c in range(N_CORES)])

    xpad = np.zeros((TOT, D), np.float32)
    xpad[newidx] = x
    x_fm = np.stack([xpad[c * NLOC:(c + 1) * NLOC].T.copy()
                     for c in range(N_CORES)])

    invcnt = (1.0 / np.maximum(counts, 1)).astype(np.float32)
    invcnt_rep = np.stack([
        np.tile(invcnt[c * GPC:(c + 1) * GPC], (128, 1)) for c in range(N_CORES)
    ]).astype(np.float32)

    return dict(
        K_lo=K_lo, K_hi=K_hi,
        NCH_lo=NCH_lo, NCH_hi=NCH_hi, PCH_lo=PCH_lo, PCH_hi=PCH_hi,
        NCHT=NCHT, NCHT_pad=NCHT_pad,
        lo_ch_off=lo_ch_off, hi_ch_off=hi_ch_off,
        idx_lo=idx_lo_w, idx_hi=idx_hi_w,
        oh_all=oh_all, diag=diag,
        x_fm=x_fm, invcnt_rep=invcnt_rep,
    )


# ===========================================================================
# device kernel
# ===========================================================================
def _build_kernel(sch):
    K_lo, K_hi = sch["K_lo"], sch["K_hi"]
    lo_ch_off, hi_ch_off = sch["lo_ch_off"], sch["hi_ch_off"]
    PCH_lo, PCH_hi = sch["PCH_lo"], sch["PCH_hi"]
    NCHT_pad = sch["NCHT_pad"]
    NCALL_lo = PCH_lo // CHUNKS_PER_CALL
    NCALL_hi = PCH_hi // CHUNKS_PER_CALL
    N_OHB = NCHT_pad // OH_BATCH

    nc = bacc.Bacc(
        "TRN2",
        target_bir_lowering=False,
        debug=False,
        num_devices=N_CORES,
        num_swdge_queues=N_QUEUES,
        dynamic_dma_scratch_size=DMA_SCRATCH,
    )

    xfm_d = nc.dram_tensor("xfm", [128, NLOC], BF16, kind="ExternalInput")
    wc_d = nc.dram_tensor("wc", [N_LAYERS, 128, 128], BF16, kind="ExternalInput")
    bct_d = nc.dram_tensor("bct", [128, N_LAYERS], F32, kind="ExternalInput")
    wffn_d = nc.dram_tensor("wffn", [256, 128], BF16, kind="ExternalInput")
    bffnt_d = nc.dram_tensor("bffnt", [128, 1], F32, kind="ExternalInput")
    wfin_d = nc.dram_tensor("wfin", [128, 2], BF16, kind="ExternalInput")
    bfinr_d = nc.dram_tensor("bfinr", [GPC, 2], F32, kind="ExternalInput")
    idxlo_d = nc.dram_tensor("idxlo", [128, PCH_lo * 8], I16, kind="ExternalInput")
    idxhi_d = nc.dram_tensor("idxhi", [128, PCH_hi * 8], I16, kind="ExternalInput")
    oh_d = nc.dram_tensor("ohall", [128, NCHT_pad, 128], BF16,
                          kind="ExternalInput")
    diag_d = nc.dram_tensor("diag", [128, NBLK, 128], BF16, kind="ExternalInput")
    invc_d = nc.dram_tensor("invc", [128, GPC], F32, kind="ExternalInput")
    ident_d = nc.dram_tensor("ident", [128, 128], BF16, kind="ExternalInput")
    out_d = nc.dram_tensor("out", [GPC, 2], F32, kind="ExternalOutput")

    RG = [list(range(N_CORES))]

    with tile.TileContext(nc) as tc:
        with (
            tc.tile_pool(name="consts", bufs=1) as consts,
            tc.tile_pool(name="hpool", bufs=2) as hpool,
            tc.tile_pool(name="zpool", bufs=2) as zpool,
            tc.tile_pool(name="zstpool", bufs=2) as zstpool,
            tc.tile_pool(name="glopool", bufs=8) as glopool,
            tc.tile_pool(name="ghipool", bufs=8) as ghipool,
            tc.tile_pool(name="ohpool", bufs=2) as ohpool,
            tc.tile_pool(name="spool", bufs=1) as spool,
            tc.tile_pool(name="ps512", bufs=2, space="PSUM") as ps512,
            tc.tile_pool(name="ps128", bufs=2, space="PSUM") as ps128,
            tc.tile_pool(name="psagg", bufs=2, space="PSUM") as psagg,
            tc.tile_pool(name="psfin", bufs=1, space="PSUM") as psfin,
            tc.tile_pool(name="dram", bufs=1, space="DRAM") as dram,
        ):
            # ---- load constants -------------------------------------------
            wc_sb = consts.tile([128, N_LAYERS, 128], BF16)
            nc.sync.dma_start(wc_sb[:], wc_d[:].rearrange("l p f -> p l f"))
            bct_sb = consts.tile([128, N_LAYERS], F32)
            nc.sync.dma_start(bct_sb[:], bct_d[:])
            wffn_sb = consts.tile([128, 2, 128], BF16)
            nc.sync.dma_start(
                wffn_sb[:], wffn_d[:].rearrange("(h p) f -> p h f", p=128))
            bffnt_sb = consts.tile([128, 1], F32)
            nc.sync.dma_start(bffnt_sb[:], bffnt_d[:])
            wfin_sb = consts.tile([128, 2], BF16)
            nc.sync.dma_start(wfin_sb[:], wfin_d[:])
            bfinr_sb = consts.tile([GPC, 2], F32)
            nc.sync.dma_start(bfinr_sb[:], bfinr_d[:])
            idxlo_sb = consts.tile([128, PCH_lo * 8], I16)
            nc.sync.dma_start(idxlo_sb[:], idxlo_d[:])
            idxhi_sb = consts.tile([128, PCH_hi * 8], I16)
            nc.sync.dma_start(idxhi_sb[:], idxhi_d[:])
            diag_sb = consts.tile([128, NBLK, 128], BF16)
            nc.sync.dma_start(diag_sb[:], diag_d[:])
            invc_sb = consts.tile([128, GPC], F32)
            nc.sync.dma_start(invc_sb[:], invc_d[:])
            ident_sb = consts.tile([128, 128], BF16)
            nc.sync.dma_start(ident_sb[:], ident_d[:])

            h_cur = hpool.tile([128, NLOC], BF16, tag="h", name="h_init")
            nc.sync.dma_start(h_cur[:], xfm_d[:])

            qctr = [0]
            gsems = [nc.alloc_semaphore(f"gdma{q}") for q in range(N_QUEUES)]

            def next_q():
                q = qctr[0] % N_QUEUES
                qctr[0] += 1
                return q

            for l in range(N_LAYERS):
                # ---- transform own slice: z[fo, n] = sum_fi Wc[fi,fo] h[fi,n]
                zst = zstpool.tile([128, NBLK, 128], BF16, tag="zst",
                                   name=f"zst{l}")
                for g in range(GPC):
                    zps = ps512.tile([128, 512], F32, tag="zps",
                                     name=f"zps{l}_{g}")
                    nc.tensor.matmul(
                        zps[:], wc_sb[:, l, :],
                        h_cur[:, g * 512:(g + 1) * 512],
                        start=True, stop=True)
                    zsb = zpool.tile([128, 512], BF16, tag="zsb",
                                     name=f"zsb{l}_{g}")
                    nc.scalar.activation(
                        zsb[:], zps[:], mybir.ActivationFunctionType.Copy)
                    for t in range(4):
                        tps = ps128.tile([128, 128], BF16, tag="tps",
                                         name=f"tps{l}_{g}_{t}")
                        nc.tensor.transpose(
                            tps[:], zsb[:, t * 128:(t + 1) * 128], ident_sb[:])
                        nc.scalar.activation(
                            zst[:, g * 4 + t, :], tps[:],
                            mybir.ActivationFunctionType.Copy)
                z_own = dram.tile([NLOC, 128], BF16, tag="zown", bufs=2,
                                  name=f"zown{l}")
                nc.sync.dma_start(
                    z_own[:].rearrange("(b p) f -> p b f", p=128), zst[:])

                # ---- AllGather the z table --------------------------------
                z_full = dram.tile([TOT, 128], BF16, tag="zfull", bufs=2,
                                   addr_space="Shared", name=f"zfull{l}")
                nc.gpsimd.collective_compute(
                    "AllGather", mybir.AluOpType.bypass,
                    replica_groups=RG,
                    ins=[z_own[:].opt()],
                    outs=[z_full[:].opt()],
                )

                # ---- gather + aggregate -----------------------------------
                h_nxt = hpool.tile([128, NLOC], BF16, tag="h", name=f"h{l + 1}")
                lo_tiles = {}
                hi_tiles = {}
                oh_tiles = {}
                lo_next = [0]
                hi_next = [0]

                def issue_lo(upto_chunk, l=l, lo_tiles=lo_tiles, lo_next=lo_next,
                             z_full=z_full):
                    while lo_next[0] * CHUNKS_PER_CALL < upto_chunk:
                        ci = lo_next[0]
                        t = glopool.tile([128, CHUNKS_PER_CALL, 128], BF16,
                                         tag="glo", name=f"glo{l}_{ci}")
                        c0 = ci * CHUNKS_PER_CALL
                        q = next_q()
                        nc.gpsimd.dma_gather(
                            t[:], z_full[0:SPLIT, :],
                            idxlo_sb[:, c0 * 8:(c0 + CHUNKS_PER_CALL) * 8],
                            num_idxs=CHUNKS_PER_CALL * 128,
                            num_idxs_reg=CHUNKS_PER_CALL * 128,
                            elem_size=128, queue_num=q,
                        )
                        lo_tiles[ci] = t
                        lo_next[0] += 1

                def issue_hi(upto_chunk, l=l, hi_tiles=hi_tiles, hi_next=hi_next,
                             z_full=z_full):
                    while hi_next[0] * CHUNKS_PER_CALL < upto_chunk:
                        ci = hi_next[0]
                        t = ghipool.tile([128, CHUNKS_PER_CALL, 128], BF16,
                                         tag="ghi", name=f"ghi{l}_{ci}")
                        c0 = ci * CHUNKS_PER_CALL
                        q = next_q()
                        nc.gpsimd.dma_gather(
                            t[:], z_full[SPLIT:TOT, :],
                            idxhi_sb[:, c0 * 8:(c0 + CHUNKS_PER_CALL) * 8],
                            num_idxs=CHUNKS_PER_CALL * 128,
                            num_idxs_reg=CHUNKS_PER_CALL * 128,
                            elem_size=128, queue_num=q,
                        )
                        hi_tiles[ci] = t
                        hi_next[0] += 1

                # interleave all lo/hi gather calls upfront: the in-order
                # gpsimd engine plus pool WAR semaphores self-pace them ~bufs
                # calls ahead of matmul consumption
                for ci in range(max(NCALL_lo, NCALL_hi)):
                    issue_lo(min((ci + 1) * CHUNKS_PER_CALL, PCH_lo))
                    issue_hi(min((ci + 1) * CHUNKS_PER_CALL, PCH_hi))
                # one-hot batch loads, upfront on the sync engine
                for k in range(N_OHB):
                    t = ohpool.tile([128, OH_BATCH, 128], BF16, tag="ohb",
                                    name=f"ohb{l}_{k}")
                    nc.sync.dma_start(
                        t[:], oh_d[:, k * OH_BATCH:(k + 1) * OH_BATCH, :])
                    oh_tiles[k] = t

                p_cons = 0      # consumption-order one-hot position
                for b in range(NBLK):
                    klo = int(K_lo[b])
                    khi = int(K_hi[b])
                    ktot = klo + khi
                    ps = psagg.tile([128, 128], F32, tag="aggps",
                                    name=f"agg{l}_{b}")
                    # self-loop contribution from local node-major z
                    nc.tensor.matmul(
                        ps[:], zst[:, b, :], diag_sb[:, b, :],
                        start=True, stop=(ktot == 0))
                    for j in range(ktot):
                        if j < klo:
                            c = int(lo_ch_off[b]) + j
                            msg = lo_tiles[c // CHUNKS_PER_CALL][
                                :, c % CHUNKS_PER_CALL, :]
                        else:
                            c = int(hi_ch_off[b]) + (j - klo)
                            msg = hi_tiles[c // CHUNKS_PER_CALL][
                                :, c % CHUNKS_PER_CALL, :]
                        oh = oh_tiles[p_cons // OH_BATCH][
                            :, p_cons % OH_BATCH, :]
                        p_cons += 1
                        nc.tensor.matmul(
                            ps[:], msg, oh,
                            start=False, stop=(j == ktot - 1))
                    nc.scalar.activation(
                        h_nxt[:, b * 128:(b + 1) * 128], ps[:],
                        mybir.ActivationFunctionType.Relu,
                        bias=bct_sb[:, l:l + 1])
                h_cur = h_nxt

            # ---- pooling + FFN --------------------------------------------
            mx = spool.tile([128, GPC], BF16)
            sm = spool.tile([128, GPC], F32)
            for g in range(GPC):
                nc.vector.tensor_reduce(
                    mx[:, g:g + 1], h_cur[:, g * GSLOT:(g + 1) * GSLOT],
                    mybir.AxisListType.X, mybir.AluOpType.max)
                nc.vector.tensor_reduce(
                    sm[:, g:g + 1], h_cur[:, g * GSLOT:(g + 1) * GSLOT],
                    mybir.AxisListType.X, mybir.AluOpType.add)
            mean = spool.tile([128, GPC], BF16)
            nc.vector.tensor_tensor(
                mean[:], sm[:], invc_sb[:], mybir.AluOpType.mult)

            p1 = psfin.tile([128, GPC], F32, tag="p1")
            nc.tensor.matmul(p1[:], wffn_sb[:, 0, :], mx[:],
                             start=True, stop=False)
            nc.tensor.matmul(p1[:], wffn_sb[:, 1, :], mean[:],
                             start=False, stop=True)
            o1 = spool.tile([128, GPC], BF16)
            nc.scalar.activation(
                o1[:], p1[:], mybir.ActivationFunctionType.Relu,
                bias=bffnt_sb[:, 0:1])

            p2 = psfin.tile([GPC, 2], F32, tag="p2")
            nc.tensor.matmul(p2[:], o1[:], wfin_sb[:], start=True, stop=True)
            osb = spool.tile([GPC, 2], F32)
            nc.vector.tensor_tensor(
                osb[:], p2[:], bfinr_sb[:], mybir.AluOpType.add)
            nc.sync.dma_start(out_d[:], osb[:])

    nc.compile()
    return nc


# ===========================================================================
# entry point
# ===========================================================================
_CACHE = {}


def kernel(x, Wc, bc, W_ffn, b_ffn, W_fin, b_fin, edge_index, batch):
    x = np.ascontiguousarray(np.asarray(x, np.float32))
    Wc = np.ascontiguousarray(np.asarray(Wc, np.float32))
    bc = np.ascontiguousarray(np.asarray(bc, np.float32))
    W_ffn = np.ascontiguousarray(np.asarray(W_ffn, np.float32))
    b_ffn = np.ascontiguousarray(np.asarray(b_ffn, np.float32))
    W_fin = np.ascontiguousarray(np.asarray(W_fin, np.float32))
    b_fin = np.ascontiguousarray(np.asarray(b_fin, np.float32))

    sch = _build_schedule(x, edge_index, batch)

    key = (sch["PCH_lo"], sch["PCH_hi"], sch["NCHT_pad"],
           tuple(sch["K_lo"]), tuple(sch["K_hi"]))
    if key not in _CACHE:
        _CACHE.clear()
        _CACHE[key] = _build_kernel(sch)
    nc = _CACHE[key]

    ident = np.eye(128, dtype=np.float32)
    bct = bc.T.copy()                       # [128, 3]
    bffnt = b_ffn[:, None].copy()           # [128, 1]
    bfinr = np.tile(b_fin[None, :], (GPC, 1)).astype(np.float32)

    def b16(a):
        return np.ascontiguousarray(a.astype(BF16NP))

    in_maps = []
    for c in range(N_CORES):
        in_maps.append({
            "xfm": b16(sch["x_fm"][c]),
            "wc": b16(Wc), "bct": bct, "wffn": b16(W_ffn), "bffnt": bffnt,
            "wfin": b16(W_fin), "bfinr": bfinr,
            "idxlo": sch["idx_lo"][c], "idxhi": sch["idx_hi"][c],
            "ohall": sch["oh_all"][c],
            "diag": b16(sch["diag"][c]),
            "invc": sch["invcnt_rep"][c],
            "ident": b16(ident),
        })

    _CACHE["in_maps"] = in_maps
    res = run_bass_kernel_spmd(nc, in_maps, core_ids=list(range(N_CORES)))
    out = np.concatenate([res.results[c]["out"] for c in range(N_CORES)], 0)
    return out.astype(np.float32)


def timed_run(inputs=None):
    """Re-run the cached compiled kernel with profiling; returns exec ns."""
    import time
    nc = next(v for k, v in _CACHE.items() if k != "in_maps")
    in_maps = _CACHE["in_maps"]
    # warm re-runs for a wall-clock floor estimate
    walls = []
    for _ in range(3):
        t0 = time.time()
        run_bass_kernel_spmd(nc, in_maps, core_ids=list(range(N_CORES)))
        walls.append(time.time() - t0)
    print(f"warm re-run walls: {[f'{w*1e3:.1f}ms' for w in walls]}")
    try:
        res = run_bass_kernel_spmd(
            nc, in_maps, core_ids=list(range(N_CORES)), trace=True)
        if res.exec_time_ns is not None:
            return res.exec_time_ns
    except Exception as e:
        print(f"(ntff profiling unavailable: {type(e).__name__}; "
              f"reporting warm wall-clock upper bound)")
    return int(min(walls) * 1e9)


if __name__ == "__main__":
    rng = np.random.default_rng(0)
    x = rng.standard_normal((N_NODES, D), dtype=np.float32)
    ei = rng.integers(0, N_NODES, (2, N_EDGES)).astype(np.int64)
    batch = np.sort(rng.integers(0, N_GRAPHS, N_NODES)).astype(np.int64)
    Wc = rng.standard_normal((3, D, D), dtype=np.float32) * 0.05
    out = kernel(x, Wc, np.zeros((3, D), np.float32),
                 rng.standard_normal((2 * D, D), dtype=np.float32) * 0.05,
                 np.zeros((D,), np.float32),
                 rng.standard_normal((D, 2), dtype=np.float32) * 0.05,
                 np.zeros((2,), np.float32), ei, batch)
    print(out.shape, out[:4])
